# revision 45
# baseline (speedup 1.0000x reference)
"""Multi-head self-attention (B=2, N=2048, D=1024, 16 heads) on 8 TRN2
NeuronCores — tensor-parallel over heads (2 heads per core), row-parallel
output projection summed on the host.

Self-contained: takes the FULL inputs of reference.setup_inputs() and
returns the FULL [2, 2048, 1024] fp32 output.

Per-core device kernel (Bass/Tile, bf16 matmuls, fp32 accumulation).
The attention phase is a software pipeline of 144 "slots" (one slot =
one 128-key tile of one 512-query unit); the ACT-engine exp
((312+1024)cyc/1.2GHz ~ 1.11us per slot) is the rate limiter, so the
schedule keeps ACT exp-only and keeps the PE's in-order queue free of
DVE-gated work ahead of the score matmuls:

  slot order: score pair (both heads, disjoint PE row groups) -> exp ->
  DVE-only norm stages of the previous unit -> AV pair (runs 2 slots
  behind scores, ones-column emits softmax denominators for free) ->
  PE norm/proj stages -> QKV drip items.

  QKV/proj work is "dripped" into per-slot PE slack as 2-ktile
  sub-chains (~1024 cyc each), deadline-scheduled (EDF with per-slot
  cycle budgets) so no slot takes a multi-us lump.  Chain partials and
  V/R/proj one-shots use two dedicated single-buffer PSUM pools so a
  chain spanning slots never blocks the rotating misc bank.

  front: weights on the gpsimd DMA ring concurrently with host-packed
  contiguous xT pieces on the sync ring (chunk-0 split by k-tile so the
  first QKV chain starts ~1MB in); a zero-matmul burst warms the PE
  clock (HAM) and a tiny exp preloads the ACT table during the DMA fill.

  norm: reciprocal_approx_fast (51 ULP, ~5x faster than the iterative
  DVE reciprocal), K=64 PE broadcast matmul to spread 1/den across both
  heads' rows, one DVE mul; projection emitted as single 512-wide
  matmuls, output DMA'd per 128-token row on alternating sync/gpsimd
  rings; the last 512-query unit is split into two 256-query halves so
  its norm+proj chain overlaps the second half instead of the tail.
"""

import sys

sys.path.insert(0, "/opt/trn_rl_repo")

import numpy as np
import ml_dtypes

import concourse.mybir as mybir
import concourse.tile as tile
import concourse.bass as bass
from concourse.bass import ts
from concourse import bass_utils
from concourse.bass_utils import run_bass_kernel_spmd

# ─────────────────────────────────────────────────────────────────────
# Environment patches (this walrus build allows only ONE sem wait per
# instruction; Tile emits several — split them into single-wait nops).
# ─────────────────────────────────────────────────────────────────────


def _patched_drain_and_barrier(self, tick_clock, wait_clock):
    from concourse.tile import ScopedClock

    nc = self.nc
    drain_inst = nc.sync.drain()
    wait_clock.add_sem_waits(
        drain_inst.ins, ScopedClock({None: tick_clock.global_clock})
    )
    waits = list(drain_inst.ins.sync_info.on_wait)
    if len(waits) > 1:
        name2sem = {}
        for k, h in self.sems.allocated().items():
            nm = getattr(h, "name", None) or str(k)
            name2sem[nm] = h
        drain_inst.ins.sync_info = mybir.SyncInfo(
            on_wait=[waits[0]], on_update=[]
        )
        for w in waits[1:]:
            h = name2sem.get(w.ant_name)
            assert h is not None, (w.ant_name, list(name2sem))
            n = nc.sync.nop(nofuse=True)
            n.wait_op(h, w.wait_value, "sem-ge")
    nc.all_engine_barrier()
    popped = nc._tile_sem_poison_stack.pop()
    assert popped is self._sem_poison
    nc.clear_and_free_semaphores(list(self.sems.allocated().values()))
    nc.all_engine_barrier()


tile.TileContext._drain_and_barrier = _patched_drain_and_barrier
bass_utils.upload_artifacts = lambda tmpdir: tmpdir

_legalize_counter = [0]


def legalize_waits(nc):
    n_split = 0
    for f in nc.m.functions:
        for bb in f.blocks:
            insts = bb.instructions
            if not any(
                i.sync_info is not None and len(i.sync_info.on_wait) > 1
                for i in insts
            ):
                continue
            new_list = []
            for ins in insts:
                si = ins.sync_info
                if si is not None and len(si.on_wait) > 1:
                    waits = list(si.on_wait)
                    for w in waits[:-1]:
                        _legalize_counter[0] += 1
                        nop = mybir.InstNoOp(
                            name=f"lw_nop_{_legalize_counter[0]}",
                            ins=[], outs=[],
                        )
                        nop.engine = ins.engine
                        nop.sync_info = mybir.SyncInfo(
                            on_wait=[w], on_update=[]
                        )
                        new_list.append(nop)
                        n_split += 1
                    ins.sync_info = mybir.SyncInfo(
                        on_wait=[waits[-1]], on_update=list(si.on_update)
                    )
                new_list.append(ins)
            bb.instructions = new_list
    return n_split


# ─────────────────────────────────────────────────────────────────────
# Kernel build
# ─────────────────────────────────────────────────────────────────────

F32 = mybir.dt.float32
BF16 = mybir.dt.bfloat16

DIM = 1024
HD = 64
B = 2
N = 2048
BN = B * N
SCALE = HD ** -0.5
N_CORES = 8
KT = DIM // 128
MC_B = N // 512   # 4 (512-query units per batch)
NT_B = N // 128   # 16 (128-key tiles per batch)
NSLOT = 144       # 7 full units x16 + 2 half units x16


def _build_nc():
    mm_dt = BF16
    nc = bass.Bass("TRN2", target_bir_lowering=False, debug=False,
                   num_devices=N_CORES)
    # host-packed activation pieces: [128p, KT, tok] so each partition's
    # DMA line is one contiguous DRAM read (kt-major inside a piece)
    xcs = [nc.dram_tensor(f"xc{mc}", [128, KT, 512], mm_dt,
                          kind="ExternalInput") for mc in range(MC_B)]
    xb1 = nc.dram_tensor("xb1", [128, KT, N], mm_dt, kind="ExternalInput")
    # weights as [p, kt*m] so the DMA moves contiguous 2KB lines
    wq = nc.dram_tensor("wq", [128, DIM], mm_dt, kind="ExternalInput")
    wk = nc.dram_tensor("wk", [128, DIM], mm_dt, kind="ExternalInput")
    wv = nc.dram_tensor("wv", [128, DIM], mm_dt, kind="ExternalInput")
    wp = nc.dram_tensor("wp", [128, DIM], mm_dt, kind="ExternalInput")
    out = nc.dram_tensor("out", [BN, DIM], BF16, kind="ExternalOutput")
    out_t = out.ap().rearrange("(c p) d -> p c d", p=128)

    with tile.TileContext(nc) as tc:
        with (
            tc.tile_pool(name="xp", bufs=1) as xp,
            tc.tile_pool(name="wpool", bufs=1) as wpool,
            tc.tile_pool(name="qk", bufs=1) as qk,
            tc.tile_pool(name="pt", bufs=8) as ptp,
            tc.tile_pool(name="small", bufs=2) as sp,
            tc.tile_pool(name="osb", bufs=2) as osbp,
            tc.tile_pool(name="ostage", bufs=3) as osp,
            tc.tile_pool(name="ps_s", bufs=2, space="PSUM") as ps_s,
            tc.tile_pool(name="ps_chain", bufs=1, space="PSUM") as ps_chain,
            tc.tile_pool(name="ps_misc", bufs=1, space="PSUM") as ps_misc,
            tc.tile_pool(name="ps_o", bufs=2, space="PSUM") as ps_o,
        ):
            # ---- engine warmups (run during the DMA fill) ----
            # tiny exp preloads the ACT spline table (~2.7us otherwise
            # paid inside the first pipeline slot)
            we_in = sp.tile([128, 64], F32, tag="warm_i", bufs=1)
            we_out = sp.tile([128, 64], BF16, tag="warm_o", bufs=1)
            nc.vector.memset(we_in[:], 0.0)
            nc.scalar.activation(we_out[:], we_in[:],
                                 mybir.ActivationFunctionType.Exp,
                                 scale=SCALE)
            # zero-matmul burst keeps the PE busy so the HAM clock gate
            # opens (1.2 -> 2.4 GHz) before the real QKV chains start
            wdum = sp.tile([128, 128], mm_dt, tag="warm_w", bufs=1)
            nc.vector.memset(wdum[:], 0.0)
            warm_ps = ps_chain.tile([128, 512], F32, tag="chain",
                                    name="warm_ps")
            for i in range(24):
                nc.tensor.matmul(warm_ps[:, 0:128], wdum[:], wdum[:],
                                 start=True, stop=True)

            # ---- input DMAs ----
            # weights on the gpsimd ring, xT pieces on the sync ring —
            # the two rings run concurrently. wp is not needed until the
            # first projection (~unit 1), so it goes after wv.
            wq_s = wpool.tile([128, KT, 128], mm_dt, tag="wq")
            wk_s = wpool.tile([128, KT, 128], mm_dt, tag="wk")
            wv_s = wpool.tile([128, KT, 128], mm_dt, tag="wv")
            wp_s = wpool.tile([128, DIM], mm_dt, tag="wp")
            for w_d, w_s in ((wq, wq_s), (wk, wk_s), (wv, wv_s)):
                nc.gpsimd.dma_start(
                    out=w_s[:],
                    in_=w_d.ap().rearrange("p (kt m) -> p kt m", kt=KT),
                )
            nc.gpsimd.dma_start(out=wp_s[:], in_=wp.ap())

            xT_s = xp.tile([128, KT, BN], mm_dt, tag="xT")
            # chunk 0 split by k-tile so the first QT chain overlaps the
            # arrival of its own operands; alternate rings (sync/vector)
            # so the pieces transfer in parallel
            for i, sl in enumerate(((0, 1), (1, 2), (2, 3), (3, 4),
                                    (4, 5), (5, 6), (6, 7), (7, KT))):
                # scalar (ACT) ring is free before the first exp
                eng = nc.sync if i % 2 == 0 else nc.scalar
                eng.dma_start(
                    out=xT_s[:, sl[0]:sl[1], 0:512],
                    in_=xcs[0].ap()[:, sl[0]:sl[1], :],
                )
            for mc in range(1, MC_B):
                nc.sync.dma_start(
                    out=xT_s[:, :, ts(mc, 512)],
                    in_=xcs[mc].ap(),
                )
            # batch 1 in two halves so its V/KT chains can start after
            # the first half lands
            nc.sync.dma_start(out=xT_s[:, :, N:N + 1024],
                              in_=xb1.ap()[:, :, 0:1024])
            nc.sync.dma_start(out=xT_s[:, :, N + 1024:BN],
                              in_=xb1.ap()[:, :, 1024:N])

            # ---- QKV work items ----
            QT_s = qk.tile([128, BN], mm_dt, tag="QT")
            KT_s = qk.tile([128, BN], mm_dt, tag="KT")
            V_s = qk.tile([128, 32, 2, 72], mm_dt, tag="V")
            nc.vector.memset(V_s[:, :, :, 64], 1.0)

            def make_chain(w_s, dst, mc, pool=None, tag="chain"):
                """QT/KT chunk as 8 sub-items of 1 k-tile (~512 cyc
                each) accumulating into one PSUM bank; the bank is
                released by the copy-out on the last sub-item."""
                st = {}
                pool = pool if pool is not None else ps_chain

                def mk(kt):
                    def f():
                        if kt == 0:
                            st["ps"] = pool.tile(
                                [128, 512], F32, tag=tag,
                                name=f"qk_ps_{mc}",
                            )
                        nc.tensor.matmul(
                            st["ps"][:], w_s[:, kt, :],
                            xT_s[:, kt, ts(mc, 512)],
                            start=(kt == 0), stop=(kt == KT - 1),
                        )
                        if kt == KT - 1:
                            nc.vector.tensor_copy(dst[:, ts(mc, 512)],
                                                  st["ps"][:])
                    return f

                return [mk(kt) for kt in range(KT)]

            def item_v(mt):
                # b=0 V tiles run inside unit 0 where the misc bank has
                # no R/proj traffic; b=1 V tiles use the chain bank
                # (chains are sparse by then, and chain blocks and V
                # items never interleave: both are contiguous runs in
                # the deadline-sorted item stream)
                pool, tg = (ps_misc, "misc") if mt < 16 else \
                    (ps_chain, "chain")

                def f():
                    ps = pool.tile([128, 2, 64], F32, tag=tg,
                                   name="v_ps")
                    for kt in range(KT):
                        nc.tensor.matmul(
                            ps[:], xT_s[:, kt, ts(mt, 128)],
                            wv_s[:, kt, :],
                            start=(kt == 0), stop=(kt == KT - 1),
                        )
                    nc.vector.tensor_copy(V_s[:, mt, :, 0:64], ps[:])
                return f

            # last 512-query unit split into two 256-query halves so its
            # norm+proj chain overlaps the second half instead of
            # serializing into the kernel tail
            units = []
            for b in range(B):
                for mc in range(MC_B):
                    u = b * MC_B + mc
                    if u < 7:
                        units.append((b, u * 512, 512))
                    else:
                        units.append((b, u * 512, 256))
                        units.append((b, u * 512 + 256, 256))

            # ---- drip schedule: EDF with per-slot cycle budgets ----
            # deadline = latest slot whose tail may emit the item.
            items = []  # (deadline, cycles, kind, fn)
            qt_chains = {mc: make_chain(wq_s, QT_s, mc)
                         for mc in range(2 * MC_B)}
            kt_chains = {mc: make_chain(wk_s, KT_s, mc)
                         for mc in range(1, 2 * MC_B)}
            # chunk 0 of Q^T and K^T interleave across the two PSUM
            # pools so both consume each xc0 k-tile piece as it lands
            kt0_chain = make_chain(wk_s, KT_s, 0, pool=ps_misc,
                                   tag="misc")
            pre_items = [f for pair in zip(qt_chains[0], kt0_chain)
                         for f in pair] + [item_v(0)]
            for j in (1, 2, 3):           # b=0 key chunks
                for s_ in kt_chains[j]:
                    items.append((max(4 * j - 2, 1), 512, "c", s_))
            for mt in range(1, 16):       # b=0 value tiles
                items.append((min(mt + 1, 15), 1024, "v", item_v(mt)))
            for u in range(1, 8):         # query chunks (6-slot margin)
                for s_ in qt_chains[u]:
                    items.append((16 * u - 6, 512, "c", s_))
            for j in (0, 1, 2, 3):        # b=1 key chunks
                for s_ in kt_chains[4 + j]:
                    items.append((59 + 4 * j, 512, "c", s_))
            for j in range(16):           # b=1 value tiles (consumed by
                # the AV pop at slot 67+j under the lag-3 AV discipline)
                items.append((63 + j, 1024, "v", item_v(16 + j)))
            items.sort(key=lambda it: it[0])

            # per-slot drip budget (PE cycles) = slot length at the ACT
            # rate minus attention minus that slot's PE-touching pending
            # stage, so drip never pushes a slot past the exp rate.
            # units[] gives per-slot widths; pendings run one unit late.
            slot_w = []
            for _, _, w_ in units:
                slot_w += [w_] * NT_B

            def slot_budget(s):
                if s < 16:
                    return 2300       # unit 0: no pending, DMA-paced
                w_ = slot_w[s]
                slot_len = 2674 if w_ == 512 else 1728
                pend = 512 if s % 16 in (0, 1, 6, 8, 9, 10, 11, 12, 13) \
                    else 0
                # mild over-commit before the b=1 crunch so chains land
                # with slack instead of stalling unit starts
                extra = 100 if s < 64 else 0
                return max(slot_len - 3 * w_ - pend + extra, 0)

            drip = [[] for _ in range(NSLOT)]
            ptr = 0
            for s in range(NSLOT):
                budget = slot_budget(s)
                used = 0
                while ptr < len(items):
                    dl, cyc, kind, fn = items[ptr]
                    if dl <= s:
                        pass          # forced: deadline reached
                    elif used + cyc > budget:
                        break
                    drip[s].append(fn)
                    used += cyc
                    ptr += 1
            assert ptr == len(items)

            # ---- attention ----
            # ones64: K=64 stationary for the per-head normalization
            # broadcast (partition bases must be 32-aligned, so the den
            # rows live at {0, 32}); row 0 -> out cols 0-63 (head 0),
            # row 32 -> out cols 64-127 (head 1), rest zero
            ones64_b = sp.tile([64, 128], mm_dt, tag="ones64", bufs=1)
            nc.vector.memset(ones64_b[:], 0.0)
            nc.vector.memset(ones64_b[0:1, 0:64], 1.0)
            nc.vector.memset(ones64_b[32:33, 64:128], 1.0)

            def norm_and_proj_stages(q_off, w, den_s, o_list, tail=False,
                                     alt=False):
                """Stages tagged (slot, position): run at that nt of the
                NEXT unit, 'pre' before / 'post' after the slot's AV
                pop.  Under the lag-3 AV discipline the previous unit's
                last AV (av15) pops at slot 2 and this unit's first AV
                (av0, writing BOTH o banks) at slot 3 — so the h0 copy
                runs post-AV at slot 2 and the h1 copy pre-AV at slot 3,
                splitting the DVE load while keeping the ps_o rotation
                race-free."""
                state = {}

                def mk_copy(h):
                    def f():
                        if h == 0:
                            state["o_sb"] = osbp.tile(
                                [128, 512], F32, tag="osb", name="o_sb")
                        if tail:
                            nc.scalar.copy(state["o_sb"][ts(h, 64), 0:w],
                                           o_list[h][0:64, 0:w])
                        else:
                            nc.vector.tensor_copy(
                                state["o_sb"][ts(h, 64), 0:w],
                                o_list[h][0:64, 0:w])
                        nc.vector.tensor_copy(
                            den_s[32 * h:32 * h + 1, 0:w],
                            o_list[h][64:65, 0:w])
                    return f

                def mk_recip(half):
                    # two half-width calls so other DVE work (chain
                    # copies that release PSUM banks) can interleave;
                    # custom-DVE reciprocal_approx_* doesn't compile on
                    # this walrus build ("ISA wrong length")
                    def f():
                        if half == 0:
                            state["r"] = sp.tile([64, 512], F32,
                                                 tag="rall", name="r_all")
                        sl = slice(half * (w // 2), (half + 1) * (w // 2))
                        nc.vector.reciprocal(state["r"][:, sl],
                                             den_s[:, sl])
                    return f

                def st_rmat():
                    rb = sp.tile([64, 512], mm_dt, tag="rb", name="r_bf")
                    nc.vector.tensor_copy(rb[:, 0:w], state["r"][:, 0:w])
                    R_ps = ps_misc.tile([128, 512], F32, tag="misc",
                                        name="R_ps")
                    nc.tensor.matmul(
                        R_ps[:, 0:w], ones64_b[:], rb[:, 0:w],
                        start=True, stop=True,
                    )
                    state["R"] = R_ps

                def st_mul():
                    AT = sp.tile([128, 512], mm_dt, tag="AT", name="AT_s")
                    nc.vector.tensor_mul(AT[:, 0:w], state["o_sb"][:, 0:w],
                                         state["R"][:, 0:w])
                    state["AT"] = AT

                def mk_proj(mt, cc):
                    def f():
                        if "out" not in state:
                            state["out"] = osp.tile(
                                [128, 4, DIM], BF16, tag="out",
                                name="out_stage",
                            )
                        if (tail or alt) and (2 * mt + cc) % 2 == 1:
                            # QKV chains/V are long done once the half
                            # units start (all deadlines < slot 112) —
                            # alternate banks so the serial MM->copy
                            # cadence pipelines instead of gating the
                            # short 720ns half-unit slots
                            p_ps = ps_chain.tile([128, 512], F32,
                                                 tag="chain", name="p_ps")
                        else:
                            p_ps = ps_misc.tile([128, 512], F32,
                                                tag="misc", name="p_ps")
                        nc.tensor.matmul(
                            p_ps[:], state["AT"][:, ts(mt, 128)],
                            wp_s[:, ts(cc, 512)],
                            start=True, stop=True,
                        )
                        if tail:
                            nc.scalar.copy(
                                state["out"][:, mt, ts(cc, 512)], p_ps[:])
                            # small pieces on both rings so the final
                            # DMA drain is short
                            for q4 in range(2 * cc, 2 * cc + 2):
                                eng = nc.sync if q4 % 2 == 0 else nc.gpsimd
                                eng.dma_start(
                                    out=out_t[:, q_off // 128 + mt,
                                              ts(q4, 256)],
                                    in_=state["out"][:, mt, ts(q4, 256)],
                                )
                        else:
                            nc.vector.tensor_copy(
                                state["out"][:, mt, ts(cc, 512)], p_ps[:])
                            if cc == 1:
                                eng = nc.sync if mt % 2 == 0 else nc.gpsimd
                                eng.dma_start(
                                    out=out_t[:, q_off // 128 + mt, :],
                                    in_=state["out"][:, mt, :],
                                )
                    return f

                nproj = 2 * (w // 128)
                # last two projs carry into slots 0-1 of the unit after
                # next (slot index 16+), giving the single misc bank's
                # ~1.28us MM->cast cadence room beyond the 8-slot window;
                # in half-width (720ns-slot) windows projs go 2 apart
                if nproj == 8:
                    pslots = [8, 9, 10, 11, 12, 13, 16, 17]
                else:
                    pslots = [8, 10, 16, 17]
                return (
                    [(2, "post", mk_copy(0)), (3, "pre", mk_copy(1)),
                     (4, "post", mk_recip(0)), (5, "post", mk_recip(1)),
                     (6, "post", st_rmat), (7, "post", st_mul)]
                    + [(pslots[k], "post", mk_proj(k // 2, k % 2))
                       for k in range(nproj)]
                )

            pending = []
            carry = []  # stages with slot >= 16 from the unit before
            av_q = []  # AV emission runs 3 slots behind scores/exp so
            # the in-order PE queue never stalls waiting on the ACT exp

            def mk_av(o_l, bb, ntt, pt, ww):
                def av():
                    for h in range(2):
                        nc.tensor.matmul(
                            o_l[h][0:65, 0:ww],
                            V_s[:, bb * NT_B + ntt, h, 0:65],
                            pt[:, h, 0:ww],
                            start=(ntt == 0),
                            stop=(ntt == NT_B - 1),
                        )
                return av

            for fn in pre_items:
                fn()

            slot = 0
            for ui, (b, q_off, w) in enumerate(units):
                is_last = ui == len(units) - 1
                alt = (not is_last) and units[ui + 1][2] == 256
                # den rows {0, 32}; memset (on the idle gpsimd engine)
                # so the unused rows can't feed inf/nan into the bf16
                # broadcast matmul
                den_s = sp.tile([64, 512], F32, tag="den")
                nc.gpsimd.memset(den_s[:], 1.0)
                o_list = [ps_o.tile([128, 512], F32, tag="o",
                                    name=f"o_ps_{h}")
                          for h in range(2)]
                for nt in range(NT_B):
                    # scores first: the exp (the pipeline rate limiter)
                    # only ever waits on this pair, never on drip work
                    s_ps = ps_s.tile([128, 2, 512], F32, tag="S")
                    for h in range(2):
                        h_sl = ts(h, 64)
                        nc.tensor.matmul(
                            s_ps[:, h, 0:w],
                            KT_s[h_sl, b * N + nt * 128:
                                 b * N + (nt + 1) * 128],
                            QT_s[h_sl, q_off:q_off + w],
                            start=True, stop=True,
                        )
                    PT_s = ptp.tile([128, 2, 512], mm_dt, tag="PT")
                    nc.scalar.activation(
                        PT_s[:, :, 0:w], s_ps[:, :, 0:w],
                        mybir.ActivationFunctionType.Exp,
                        scale=SCALE,
                    )
                    for sl_, pos, fn in pending:
                        if sl_ == nt and pos == "pre":
                            fn()
                    av_q.append(mk_av(o_list, b, nt, PT_s, w))
                    # in the last unit drain the AV backlog early so the
                    # tail norm chain starts sooner
                    lag = 1 if (is_last and nt >= 13) else 3
                    while len(av_q) > lag:
                        av_q.pop(0)()
                    # drip before the post stages so the chain/V copies
                    # that release PSUM banks sit early in the DVE queue
                    for fn in drip[slot]:
                        fn()
                    for sl_, _, fn in carry:
                        if sl_ == 16 + nt:
                            fn()
                    for sl_, pos, fn in pending:
                        if sl_ == nt and pos == "post":
                            fn()
                    slot += 1
                carry = [st for st in pending if st[0] >= 16]
                pending = norm_and_proj_stages(
                    q_off, w, den_s, o_list, tail=(q_off + w == BN),
                    alt=alt)
            while av_q:
                av_q.pop(0)()
            # `carry` now holds the second-to-last unit's carry-slot
            # projs (reassigned after the last unit's slots ran, so they
            # never popped); the last unit's stages were never popped
            # at all — run both.
            for _, _, fn in carry:
                fn()
            for _, _, fn in pending:
                fn()
    legalize_waits(nc)
    return nc


_CACHE = {}


def _get_nc():
    if "nc" not in _CACHE:
        _CACHE["nc"] = _build_nc()
    return _CACHE["nc"]


# ─────────────────────────────────────────────────────────────────────
# Host-side packing
# ─────────────────────────────────────────────────────────────────────

def wpack_test(w):
    # [DIM, 128] -> [128p, KT*128] so each SBUF partition line is one
    # contiguous 2KB DMA read
    return np.ascontiguousarray(
        np.asarray(w, dtype=np.float32)
        .reshape(KT, 128, 128).transpose(1, 0, 2).reshape(128, DIM)
    ).astype(ml_dtypes.bfloat16)


def xpack_test(x):
    """Full x [B, N, DIM] -> dict of contiguous bf16 DMA pieces in
    [128p, KT, tok] layout (per-partition lines are contiguous DRAM)."""
    bf = ml_dtypes.bfloat16
    xT = np.asarray(x, dtype=np.float32).reshape(BN, DIM).T  # [DIM, BN]
    pieces = {}
    for mc in range(MC_B):
        pieces[f"xc{mc}"] = np.ascontiguousarray(
            xT[:, mc * 512:(mc + 1) * 512]
            .reshape(KT, 128, 512).transpose(1, 0, 2)
        ).astype(bf)
    pieces["xb1"] = np.ascontiguousarray(
        xT[:, N:BN].reshape(KT, 128, N).transpose(1, 0, 2)
    ).astype(bf)
    return pieces


def build_in_maps(x, w_qkv, w_proj):
    """Per-core input maps (shared xT pieces, per-core weight slices)."""
    w_qkv = np.asarray(w_qkv, dtype=np.float32)
    w_proj = np.asarray(w_proj, dtype=np.float32)
    xp = xpack_test(x)
    bf = ml_dtypes.bfloat16
    in_maps = []
    for c in range(N_CORES):
        sl = slice(128 * c, 128 * (c + 1))
        in_maps.append({
            **xp,
            "wq": wpack_test(w_qkv[:, sl]),
            "wk": wpack_test(w_qkv[:, DIM + 128 * c:DIM + 128 * (c + 1)]),
            "wv": wpack_test(
                w_qkv[:, 2 * DIM + 128 * c:2 * DIM + 128 * (c + 1)]),
            "wp": np.ascontiguousarray(w_proj[sl, :]).astype(bf),
        })
    return in_maps


def kernel(x, w_qkv, w_proj, b_proj):
    x = np.asarray(x, dtype=np.float32)
    b_proj = np.asarray(b_proj, dtype=np.float32)

    nc = _get_nc()
    in_maps = build_in_maps(x, w_qkv, w_proj)
    res = run_bass_kernel_spmd(nc, in_maps, list(range(N_CORES)),
                               trace=False)
    acc = res.results[0]["out"].astype(np.float32).copy()
    for c in range(1, N_CORES):
        acc += res.results[c]["out"]
    acc += b_proj[None, :]
    return acc.reshape(B, N, DIM)


# revision 48
# speedup vs baseline: 1.0042x; 1.0042x over previous
"""Multi-head self-attention (B=2, N=2048, D=1024, 16 heads) on 8 TRN2
NeuronCores — tensor-parallel over heads (2 heads per core), row-parallel
output projection summed on the host.

Self-contained: takes the FULL inputs of reference.setup_inputs() and
returns the FULL [2, 2048, 1024] fp32 output.

Per-core device kernel (Bass/Tile, bf16 matmuls, fp32 accumulation).
The attention phase is a software pipeline of 144 "slots" (one slot =
one 128-key tile of one 512-query unit); the ACT-engine exp
((312+1024)cyc/1.2GHz ~ 1.11us per slot) is the rate limiter, so the
schedule keeps ACT exp-only and keeps the PE's in-order queue free of
slow-gated work ahead of the score matmuls:

  slot order: score pair (both heads, disjoint PE row groups; emitted
  first so the exp only ever waits on it) -> exp -> previous unit's
  norm stages -> AV pair (lag 3, ones-column emits softmax
  denominators for free) -> drip items -> PE norm/proj stages.

  QKV/proj/V/R share one 2-buffer PSUM pool (ps_big) so every
  MM->copy->MM sequence has a full slot of slack; QT/KT chunks drip as
  two 4-ktile halves in adjacent slots (a chunk tile may only see ONE
  other pool allocation while open, which the scheduler guarantees by
  keeping V items out of half-A slots and emitting half-B first in its
  slot).  Items are deadline-scheduled (EDF) with per-slot budgets.

  norm stages of unit u pop at fixed slots of unit u+1: o_sb copy h0 @2
  (after the previous unit's last AV retires the o banks), h1 @3
  (before this unit's first AV pop), reciprocal halves @4,5, 1/den
  broadcast matmul @6, o*R mul @7, eight single-MM projections @8..15.

  front: weights on the gpsimd DMA ring; host-packed contiguous xT
  pieces on the sync+scalar rings (chunk 0 per-ktile, alternating);
  Q^T/K^T chunk-0 chains interleave on the two pool buffers so both
  consume each piece as it lands; a zero-matmul burst warms the PE
  clock (HAM) and a tiny exp preloads the ACT table during the fill.

  tail: last 512-query unit split into two 256-query halves so its
  norm+proj chain overlaps the second half; AV lag drops to 1 for the
  last slots; final output DMAs go out in 64KB pieces on alternating
  sync/gpsimd rings so the post-kernel drain is short.
"""

import sys

sys.path.insert(0, "/opt/trn_rl_repo")

import numpy as np
import ml_dtypes

import concourse.mybir as mybir
import concourse.tile as tile
import concourse.bass as bass
from concourse.bass import ts
from concourse import bass_utils
from concourse.bass_utils import run_bass_kernel_spmd

# ─────────────────────────────────────────────────────────────────────
# Environment patches (this walrus build allows only ONE sem wait per
# instruction; Tile emits several — split them into single-wait nops).
# ─────────────────────────────────────────────────────────────────────


def _patched_drain_and_barrier(self, tick_clock, wait_clock):
    from concourse.tile import ScopedClock

    nc = self.nc
    drain_inst = nc.sync.drain()
    wait_clock.add_sem_waits(
        drain_inst.ins, ScopedClock({None: tick_clock.global_clock})
    )
    waits = list(drain_inst.ins.sync_info.on_wait)
    if len(waits) > 1:
        name2sem = {}
        for k, h in self.sems.allocated().items():
            nm = getattr(h, "name", None) or str(k)
            name2sem[nm] = h
        drain_inst.ins.sync_info = mybir.SyncInfo(
            on_wait=[waits[0]], on_update=[]
        )
        for w in waits[1:]:
            h = name2sem.get(w.ant_name)
            assert h is not None, (w.ant_name, list(name2sem))
            n = nc.sync.nop(nofuse=True)
            n.wait_op(h, w.wait_value, "sem-ge")
    nc.all_engine_barrier()
    popped = nc._tile_sem_poison_stack.pop()
    assert popped is self._sem_poison
    nc.clear_and_free_semaphores(list(self.sems.allocated().values()))
    nc.all_engine_barrier()


tile.TileContext._drain_and_barrier = _patched_drain_and_barrier
bass_utils.upload_artifacts = lambda tmpdir: tmpdir

_legalize_counter = [0]


def legalize_waits(nc):
    n_split = 0
    for f in nc.m.functions:
        for bb in f.blocks:
            insts = bb.instructions
            if not any(
                i.sync_info is not None and len(i.sync_info.on_wait) > 1
                for i in insts
            ):
                continue
            new_list = []
            for ins in insts:
                si = ins.sync_info
                if si is not None and len(si.on_wait) > 1:
                    waits = list(si.on_wait)
                    for w in waits[:-1]:
                        _legalize_counter[0] += 1
                        nop = mybir.InstNoOp(
                            name=f"lw_nop_{_legalize_counter[0]}",
                            ins=[], outs=[],
                        )
                        nop.engine = ins.engine
                        nop.sync_info = mybir.SyncInfo(
                            on_wait=[w], on_update=[]
                        )
                        new_list.append(nop)
                        n_split += 1
                    ins.sync_info = mybir.SyncInfo(
                        on_wait=[waits[-1]], on_update=list(si.on_update)
                    )
                new_list.append(ins)
            bb.instructions = new_list
    return n_split


# ─────────────────────────────────────────────────────────────────────
# Kernel build
# ─────────────────────────────────────────────────────────────────────

F32 = mybir.dt.float32
BF16 = mybir.dt.bfloat16

DIM = 1024
HD = 64
B = 2
N = 2048
BN = B * N
SCALE = HD ** -0.5
N_CORES = 8
KT = DIM // 128
MC_B = N // 512   # 4 (512-query units per batch)
NT_B = N // 128   # 16 (128-key tiles per batch)
NSLOT = 144       # 7 full units x16 + 2 half units x16


def _build_nc():
    mm_dt = BF16
    nc = bass.Bass("TRN2", target_bir_lowering=False, debug=False,
                   num_devices=N_CORES)
    # host-packed activation pieces: [128p, KT, tok] so each partition's
    # DMA line is one contiguous DRAM read (kt-major inside a piece)
    xcs = [nc.dram_tensor(f"xc{mc}", [128, KT, 512], mm_dt,
                          kind="ExternalInput") for mc in range(MC_B)]
    xb1 = nc.dram_tensor("xb1", [128, KT, N], mm_dt, kind="ExternalInput")
    # weights as [p, kt*m] so the DMA moves contiguous 2KB lines
    wq = nc.dram_tensor("wq", [128, DIM], mm_dt, kind="ExternalInput")
    wk = nc.dram_tensor("wk", [128, DIM], mm_dt, kind="ExternalInput")
    wv = nc.dram_tensor("wv", [128, DIM], mm_dt, kind="ExternalInput")
    wp = nc.dram_tensor("wp", [128, DIM], mm_dt, kind="ExternalInput")
    out = nc.dram_tensor("out", [BN, DIM], BF16, kind="ExternalOutput")
    out_t = out.ap().rearrange("(c p) d -> p c d", p=128)

    with tile.TileContext(nc) as tc:
        with (
            tc.tile_pool(name="xp", bufs=1) as xp,
            tc.tile_pool(name="wpool", bufs=1) as wpool,
            tc.tile_pool(name="qk", bufs=1) as qk,
            tc.tile_pool(name="pt", bufs=8) as ptp,
            tc.tile_pool(name="small", bufs=2) as sp,
            tc.tile_pool(name="osb", bufs=2) as osbp,
            tc.tile_pool(name="ostage", bufs=3) as osp,
            tc.tile_pool(name="ps_s", bufs=2, space="PSUM") as ps_s,
            tc.tile_pool(name="ps_big", bufs=2, space="PSUM") as ps_big,
            tc.tile_pool(name="ps_o", bufs=2, space="PSUM") as ps_o,
        ):
            # ---- engine warmups (run during the DMA fill) ----
            # tiny exp preloads the ACT spline table (~2.7us otherwise
            # paid inside the first pipeline slot)
            we_in = sp.tile([128, 64], F32, tag="warm_i", bufs=1)
            we_out = sp.tile([128, 64], BF16, tag="warm_o", bufs=1)
            nc.vector.memset(we_in[:], 0.0)
            nc.scalar.activation(we_out[:], we_in[:],
                                 mybir.ActivationFunctionType.Exp,
                                 scale=SCALE)
            # zero-matmul burst keeps the PE busy so the HAM clock gate
            # opens (1.2 -> 2.4 GHz) before the real QKV chains start
            wdum = sp.tile([128, 128], mm_dt, tag="warm_w", bufs=1)
            nc.vector.memset(wdum[:], 0.0)
            warm_ps = ps_big.tile([128, 512], F32, tag="big",
                                  name="warm_ps")
            for i in range(24):
                nc.tensor.matmul(warm_ps[:, 0:128], wdum[:], wdum[:],
                                 start=True, stop=True)

            # ---- input DMAs ----
            # weights on the gpsimd ring; xT pieces split between the
            # sync and scalar (ACT is idle pre-exp) rings. wp isn't
            # needed until the first projection, so it goes last.
            wq_s = wpool.tile([128, KT, 128], mm_dt, tag="wq")
            wk_s = wpool.tile([128, KT, 128], mm_dt, tag="wk")
            wv_s = wpool.tile([128, KT, 128], mm_dt, tag="wv")
            wp_s = wpool.tile([128, DIM], mm_dt, tag="wp")
            for w_d, w_s in ((wq, wq_s), (wk, wk_s), (wv, wv_s)):
                nc.gpsimd.dma_start(
                    out=w_s[:],
                    in_=w_d.ap().rearrange("p (kt m) -> p kt m", kt=KT),
                )
            nc.gpsimd.dma_start(out=wp_s[:], in_=wp.ap())

            xT_s = xp.tile([128, KT, BN], mm_dt, tag="xT")
            # chunk 0 per k-tile, alternating rings, so the first QT/KT
            # chains overlap the arrival of their own operands
            for kt in range(KT):
                eng = nc.sync if kt % 2 == 0 else nc.scalar
                eng.dma_start(
                    out=xT_s[:, kt:kt + 1, 0:512],
                    in_=xcs[0].ap()[:, kt:kt + 1, :],
                )
            for mc in range(1, MC_B):
                nc.sync.dma_start(
                    out=xT_s[:, :, ts(mc, 512)],
                    in_=xcs[mc].ap(),
                )
            # batch 1 in two halves so its V/KT chains can start after
            # the first half lands
            nc.sync.dma_start(out=xT_s[:, :, N:N + 1024],
                              in_=xb1.ap()[:, :, 0:1024])
            nc.sync.dma_start(out=xT_s[:, :, N + 1024:BN],
                              in_=xb1.ap()[:, :, 1024:N])

            # ---- QKV work items ----
            QT_s = qk.tile([128, BN], mm_dt, tag="QT")
            KT_s = qk.tile([128, BN], mm_dt, tag="KT")
            V_s = qk.tile([128, 32, 2, 72], mm_dt, tag="V")
            nc.vector.memset(V_s[:, :, :, 64], 1.0)

            def make_chain(w_s, dst, mc, nsub=2):
                """QT/KT chunk as nsub sub-items accumulating into one
                ps_big tile; released by the copy on the last sub."""
                st = {}
                kts = [range(i * KT // nsub, (i + 1) * KT // nsub)
                       for i in range(nsub)]

                def mk(i):
                    def f():
                        if i == 0:
                            st["ps"] = ps_big.tile(
                                [128, 512], F32, tag="big",
                                name=f"qk_ps_{mc}",
                            )
                        for kt in kts[i]:
                            nc.tensor.matmul(
                                st["ps"][:], w_s[:, kt, :],
                                xT_s[:, kt, ts(mc, 512)],
                                start=(kt == 0), stop=(kt == KT - 1),
                            )
                        if i == nsub - 1:
                            nc.vector.tensor_copy(dst[:, ts(mc, 512)],
                                                  st["ps"][:])
                    return f

                return [mk(i) for i in range(nsub)]

            def item_v(mt):
                def f():
                    ps = ps_big.tile([128, 2, 64], F32, tag="big",
                                     name="v_ps")
                    for kt in range(KT):
                        nc.tensor.matmul(
                            ps[:], xT_s[:, kt, ts(mt, 128)],
                            wv_s[:, kt, :],
                            start=(kt == 0), stop=(kt == KT - 1),
                        )
                    nc.vector.tensor_copy(V_s[:, mt, :, 0:64], ps[:])
                return f

            # last 512-query unit split into two 256-query halves so its
            # norm+proj chain overlaps the second half instead of
            # serializing into the kernel tail
            units = []
            for b in range(B):
                for mc in range(MC_B):
                    u = b * MC_B + mc
                    if u < 7:
                        units.append((b, u * 512, 512))
                    else:
                        units.append((b, u * 512, 256))
                        units.append((b, u * 512 + 256, 256))

            # ---- drip schedule ----
            # EDF with per-slot budgets.  A chunk's two halves go in
            # adjacent slots: half-A last in its slot (with no V item
            # there) and half-B first in the next, so the open ps_big
            # tile sees at most one other allocation (the slot's proj/R)
            # while live — safe under the 2-buffer rotation.
            items = []  # (deadline, kind, payload)
            qt_chains = {mc: make_chain(wq_s, QT_s, mc)
                         for mc in range(2 * MC_B)}
            kt_chains = {mc: make_chain(wk_s, KT_s, mc)
                         for mc in range(1, 2 * MC_B)}
            # chunk 0 of Q^T and K^T interleave across the two pool
            # buffers so both consume each xc0 k-tile piece as it lands
            qt0 = make_chain(wq_s, QT_s, 0, nsub=KT)
            kt0 = make_chain(wk_s, KT_s, 0, nsub=KT)
            pre_items = [f for pair in zip(qt0, kt0) for f in pair] \
                + [item_v(0)]
            for j in (1, 2, 3):           # b=0 key chunks
                items.append((max(4 * j - 2, 1), "c", kt_chains[j]))
            for mt in range(1, 16):       # b=0 value tiles
                items.append((min(mt + 1, 15), "v", item_v(mt)))
            for u in range(1, 8):         # query chunks (4-slot margin)
                items.append((16 * u - 4, "c", qt_chains[u]))
            for j in (0, 1, 2, 3):        # b=1 key chunks
                items.append((60 + 4 * j, "c", kt_chains[4 + j]))
            for j in range(16):           # b=1 value tiles (consumed by
                # the AV pop at slot 67+j under the lag-3 AV discipline)
                items.append((64 + j, "v", item_v(16 + j)))
            items.sort(key=lambda it: it[0])

            slot_w = []
            for _, _, w_ in units:
                slot_w += [w_] * NT_B

            def slot_budget(s):
                if s < 16:
                    return 3100       # unit 0: no pending, DMA-paced
                w_ = slot_w[s]
                slot_len = 2674 if w_ == 512 else 1728
                pend = 512 if s % 16 in (6, 8, 9, 10, 11, 12, 13, 14, 15) \
                    else 0
                return max(slot_len - 3 * w_ - pend + 150, 0)

            drip = [[] for _ in range(NSLOT)]
            no_v = [False] * NSLOT
            used = [0] * NSLOT
            ptr = 0
            for s in range(NSLOT):
                v_ok = not no_v[s]
                while ptr < len(items):
                    dl, kind, payload = items[ptr]
                    if kind == "v":
                        cyc = 1024
                        if not v_ok:
                            if dl > s:
                                break
                            # forced into a half-A slot: insert before
                            # the open chunk's half-A (the previous
                            # chunk, if any, is already closed)
                            drip[s].insert(len(drip[s]) - 1, payload)
                            used[s] += cyc
                            ptr += 1
                            continue
                        if dl > s and used[s] + cyc > slot_budget(s):
                            break
                        drip[s].append(payload)
                        used[s] += cyc
                    else:
                        # chunk halves: A here (and no V after it),
                        # B first in the next slot
                        if s + 1 >= NSLOT:
                            raise AssertionError("chunk spills past end")
                        if dl > s and (used[s] + 2048 > slot_budget(s)
                                       or used[s + 1] + 2048
                                       > slot_budget(s + 1)):
                            break
                        drip[s].append(payload[0])
                        used[s] += 2048
                        drip[s + 1].insert(0, payload[1])
                        used[s + 1] += 2048
                        no_v[s] = True
                        ptr += 1
                        break  # at most one half-A per slot
                    ptr += 1
            assert ptr == len(items), (ptr, len(items))

            # ---- attention ----
            # ones64: K=64 stationary for the per-head normalization
            # broadcast (partition bases must be 32-aligned, so the den
            # rows live at {0, 32}); row 0 -> out cols 0-63 (head 0),
            # row 32 -> out cols 64-127 (head 1), rest zero
            ones64_b = sp.tile([64, 128], mm_dt, tag="ones64", bufs=1)
            nc.vector.memset(ones64_b[:], 0.0)
            nc.vector.memset(ones64_b[0:1, 0:64], 1.0)
            nc.vector.memset(ones64_b[32:33, 64:128], 1.0)

            def norm_and_proj_stages(q_off, w, den_s, o_list, tail=False):
                """Stages tagged (slot, position): run at that nt of the
                NEXT unit, 'pre' before / 'post' after the slot's AV
                pop.  Under the lag-3 AV discipline the previous unit's
                last AV (av15) pops at slot 2 and this unit's first AV
                (av0, writing BOTH o banks) at slot 3 — so the h0 copy
                runs post-AV at slot 2 and the h1 copy pre-AV at slot
                3."""
                state = {}

                def mk_copy(h):
                    def f():
                        if h == 0:
                            state["o_sb"] = osbp.tile(
                                [128, 512], F32, tag="osb", name="o_sb")
                        if tail:
                            nc.scalar.copy(state["o_sb"][ts(h, 64), 0:w],
                                           o_list[h][0:64, 0:w])
                        else:
                            nc.vector.tensor_copy(
                                state["o_sb"][ts(h, 64), 0:w],
                                o_list[h][0:64, 0:w])
                        nc.vector.tensor_copy(
                            den_s[32 * h:32 * h + 1, 0:w],
                            o_list[h][64:65, 0:w])
                    return f

                def mk_recip(half):
                    # two half-width calls so other DVE work (the pool
                    # release copies) can interleave; the custom-DVE
                    # reciprocal_approx_* ops don't compile on this
                    # walrus build ("ISA wrong length")
                    def f():
                        if half == 0:
                            state["r"] = sp.tile([64, 512], F32,
                                                 tag="rall", name="r_all")
                        sl = slice(half * (w // 2), (half + 1) * (w // 2))
                        nc.vector.reciprocal(state["r"][:, sl],
                                             den_s[:, sl])
                    return f

                def st_rmat():
                    rb = sp.tile([64, 512], mm_dt, tag="rb", name="r_bf")
                    nc.vector.tensor_copy(rb[:, 0:w], state["r"][:, 0:w])
                    R_ps = ps_big.tile([128, 512], F32, tag="big",
                                       name="R_ps")
                    nc.tensor.matmul(
                        R_ps[:, 0:w], ones64_b[:], rb[:, 0:w],
                        start=True, stop=True,
                    )
                    state["R"] = R_ps

                def st_mul():
                    AT = sp.tile([128, 512], mm_dt, tag="AT", name="AT_s")
                    nc.vector.tensor_mul(AT[:, 0:w], state["o_sb"][:, 0:w],
                                         state["R"][:, 0:w])
                    state["AT"] = AT

                def mk_proj(mt, cc):
                    def f():
                        if "out" not in state:
                            state["out"] = osp.tile(
                                [128, 4, DIM], BF16, tag="out",
                                name="out_stage",
                            )
                        p_ps = ps_big.tile([128, 512], F32, tag="big",
                                           name="p_ps")
                        nc.tensor.matmul(
                            p_ps[:], state["AT"][:, ts(mt, 128)],
                            wp_s[:, ts(cc, 512)],
                            start=True, stop=True,
                        )
                        if tail:
                            nc.scalar.copy(
                                state["out"][:, mt, ts(cc, 512)], p_ps[:])
                            # small pieces on both rings so the final
                            # DMA drain is short
                            for q4 in range(2 * cc, 2 * cc + 2):
                                eng = nc.sync if q4 % 2 == 0 \
                                    else nc.gpsimd
                                eng.dma_start(
                                    out=out_t[:, q_off // 128 + mt,
                                              ts(q4, 256)],
                                    in_=state["out"][:, mt, ts(q4, 256)],
                                )
                        else:
                            nc.vector.tensor_copy(
                                state["out"][:, mt, ts(cc, 512)], p_ps[:])
                            if cc == 1:
                                eng = nc.sync if mt % 2 == 0 \
                                    else nc.gpsimd
                                eng.dma_start(
                                    out=out_t[:, q_off // 128 + mt, :],
                                    in_=state["out"][:, mt, :],
                                )
                    return f

                return (
                    [(2, "post", mk_copy(0)), (3, "pre", mk_copy(1)),
                     (4, "post", mk_recip(0)), (5, "post", mk_recip(1)),
                     (6, "post", st_rmat), (7, "post", st_mul)]
                    + [(8 + k, "post", mk_proj(k // 2, k % 2))
                       for k in range(2 * (w // 128))]
                )

            pending = []
            av_q = []  # AV emission runs 3 slots behind scores/exp so
            # the in-order PE queue never stalls waiting on the ACT exp

            def mk_av(o_l, bb, ntt, pt, ww):
                def av():
                    for h in range(2):
                        nc.tensor.matmul(
                            o_l[h][0:65, 0:ww],
                            V_s[:, bb * NT_B + ntt, h, 0:65],
                            pt[:, h, 0:ww],
                            start=(ntt == 0),
                            stop=(ntt == NT_B - 1),
                        )
                return av

            for fn in pre_items:
                fn()

            slot = 0
            for ui, (b, q_off, w) in enumerate(units):
                is_last = ui == len(units) - 1
                # den rows {0, 32}; memset (on the idle gpsimd engine)
                # so the unused rows can't feed inf/nan into the bf16
                # broadcast matmul
                den_s = sp.tile([64, 512], F32, tag="den")
                nc.gpsimd.memset(den_s[:], 1.0)
                o_list = [ps_o.tile([128, 512], F32, tag="o",
                                    name=f"o_ps_{h}")
                          for h in range(2)]
                for nt in range(NT_B):
                    # scores first: the exp (the pipeline rate limiter)
                    # only ever waits on this pair, never on drip work
                    s_ps = ps_s.tile([128, 2, 512], F32, tag="S")
                    for h in range(2):
                        h_sl = ts(h, 64)
                        nc.tensor.matmul(
                            s_ps[:, h, 0:w],
                            KT_s[h_sl, b * N + nt * 128:
                                 b * N + (nt + 1) * 128],
                            QT_s[h_sl, q_off:q_off + w],
                            start=True, stop=True,
                        )
                    PT_s = ptp.tile([128, 2, 512], mm_dt, tag="PT")
                    nc.scalar.activation(
                        PT_s[:, :, 0:w], s_ps[:, :, 0:w],
                        mybir.ActivationFunctionType.Exp,
                        scale=SCALE,
                    )
                    for sl_, pos, fn in pending:
                        if sl_ == nt and pos == "pre":
                            fn()
                    av_q.append(mk_av(o_list, b, nt, PT_s, w))
                    # in the last unit drain the AV backlog early so the
                    # tail norm chain starts sooner
                    lag = 1 if (is_last and nt >= 13) else 3
                    while len(av_q) > lag:
                        av_q.pop(0)()
                    # drip before the post stages so the pool-release
                    # copies sit early in the DVE queue
                    for fn in drip[slot]:
                        fn()
                    for sl_, pos, fn in pending:
                        if sl_ == nt and pos == "post":
                            fn()
                    slot += 1
                pending = norm_and_proj_stages(
                    q_off, w, den_s, o_list, tail=(q_off + w == BN))
            while av_q:
                av_q.pop(0)()
            for _, _, fn in pending:
                fn()
    legalize_waits(nc)
    return nc


_CACHE = {}


def _get_nc():
    if "nc" not in _CACHE:
        _CACHE["nc"] = _build_nc()
    return _CACHE["nc"]


# ─────────────────────────────────────────────────────────────────────
# Host-side packing
# ─────────────────────────────────────────────────────────────────────

def wpack_test(w):
    # [DIM, 128] -> [128p, KT*128] so each SBUF partition line is one
    # contiguous 2KB DMA read
    return np.ascontiguousarray(
        np.asarray(w, dtype=np.float32)
        .reshape(KT, 128, 128).transpose(1, 0, 2).reshape(128, DIM)
    ).astype(ml_dtypes.bfloat16)


def xpack_test(x):
    """Full x [B, N, DIM] -> dict of contiguous bf16 DMA pieces in
    [128p, KT, tok] layout (per-partition lines are contiguous DRAM)."""
    bf = ml_dtypes.bfloat16
    xT = np.asarray(x, dtype=np.float32).reshape(BN, DIM).T  # [DIM, BN]
    pieces = {}
    for mc in range(MC_B):
        pieces[f"xc{mc}"] = np.ascontiguousarray(
            xT[:, mc * 512:(mc + 1) * 512]
            .reshape(KT, 128, 512).transpose(1, 0, 2)
        ).astype(bf)
    pieces["xb1"] = np.ascontiguousarray(
        xT[:, N:BN].reshape(KT, 128, N).transpose(1, 0, 2)
    ).astype(bf)
    return pieces


def build_in_maps(x, w_qkv, w_proj):
    """Per-core input maps (shared xT pieces, per-core weight slices)."""
    w_qkv = np.asarray(w_qkv, dtype=np.float32)
    w_proj = np.asarray(w_proj, dtype=np.float32)
    xp = xpack_test(x)
    bf = ml_dtypes.bfloat16
    in_maps = []
    for c in range(N_CORES):
        sl = slice(128 * c, 128 * (c + 1))
        in_maps.append({
            **xp,
            "wq": wpack_test(w_qkv[:, sl]),
            "wk": wpack_test(w_qkv[:, DIM + 128 * c:DIM + 128 * (c + 1)]),
            "wv": wpack_test(
                w_qkv[:, 2 * DIM + 128 * c:2 * DIM + 128 * (c + 1)]),
            "wp": np.ascontiguousarray(w_proj[sl, :]).astype(bf),
        })
    return in_maps


def kernel(x, w_qkv, w_proj, b_proj):
    x = np.asarray(x, dtype=np.float32)
    b_proj = np.asarray(b_proj, dtype=np.float32)

    nc = _get_nc()
    in_maps = build_in_maps(x, w_qkv, w_proj)
    res = run_bass_kernel_spmd(nc, in_maps, list(range(N_CORES)),
                               trace=False)
    acc = res.results[0]["out"].astype(np.float32).copy()
    for c in range(1, N_CORES):
        acc += res.results[c]["out"]
    acc += b_proj[None, :]
    return acc.reshape(B, N, DIM)


# revision 50
# speedup vs baseline: 1.0349x; 1.0306x over previous
"""Multi-head self-attention (B=2, N=2048, D=1024, 16 heads) on 8 TRN2
NeuronCores — tensor-parallel over heads (2 heads per core), row-parallel
output projection summed on the host.

Self-contained: takes the FULL inputs of reference.setup_inputs() and
returns the FULL [2, 2048, 1024] fp32 output.

Per-core device kernel (Bass/Tile, bf16 matmuls, fp32 accumulation).
The attention phase is a software pipeline of 144 "slots" (one slot =
one 128-key tile of one 512-query unit); the ACT-engine exp
((312+1024)cyc/1.2GHz ~ 1.11us per slot) is the rate limiter, so the
schedule keeps ACT exp-only and keeps the PE's in-order queue free of
slow-gated work ahead of the score matmuls:

  slot order: score pair (both heads, disjoint PE row groups; emitted
  first so the exp only ever waits on it) -> exp -> previous unit's
  norm stages -> AV pair (lag 3, ones-column emits softmax
  denominators for free) -> drip items -> PE norm/proj stages.

  QKV/proj/V/R share one 2-buffer PSUM pool (ps_big) so every
  MM->copy->MM sequence has a full slot of slack; QT/KT chunks drip as
  two 4-ktile halves in adjacent slots (a chunk tile may only see ONE
  other pool allocation while open, which the scheduler guarantees by
  keeping V items out of half-A slots and emitting half-B first in its
  slot).  Items are deadline-scheduled (EDF) with per-slot budgets.

  norm stages of unit u pop at fixed slots of unit u+1: o_sb copy h0 @2
  (after the previous unit's last AV retires the o banks), h1 @3
  (before this unit's first AV pop), reciprocal halves @4,5, 1/den
  broadcast matmul @6, o*R mul @7, eight single-MM projections @8..15.

  front: weights on the gpsimd DMA ring; host-packed contiguous xT
  pieces on the sync+scalar rings (chunk 0 per-ktile, alternating);
  Q^T/K^T chunk-0 chains interleave on the two pool buffers so both
  consume each piece as it lands; a zero-matmul burst warms the PE
  clock (HAM) and a tiny exp preloads the ACT table during the fill.

  tail: last 512-query unit split into two 256-query halves so its
  norm+proj chain overlaps the second half; AV lag drops to 1 for the
  last slots; final output DMAs go out in 64KB pieces on alternating
  sync/gpsimd rings so the post-kernel drain is short.
"""

import sys

sys.path.insert(0, "/opt/trn_rl_repo")

import numpy as np
import ml_dtypes

import concourse.mybir as mybir
import concourse.tile as tile
import concourse.bass as bass
from concourse.bass import ts
from concourse import bass_utils
from concourse.bass_utils import run_bass_kernel_spmd

# ─────────────────────────────────────────────────────────────────────
# Environment patches (this walrus build allows only ONE sem wait per
# instruction; Tile emits several — split them into single-wait nops).
# ─────────────────────────────────────────────────────────────────────


def _patched_drain_and_barrier(self, tick_clock, wait_clock):
    from concourse.tile import ScopedClock

    nc = self.nc
    drain_inst = nc.sync.drain()
    wait_clock.add_sem_waits(
        drain_inst.ins, ScopedClock({None: tick_clock.global_clock})
    )
    waits = list(drain_inst.ins.sync_info.on_wait)
    if len(waits) > 1:
        name2sem = {}
        for k, h in self.sems.allocated().items():
            nm = getattr(h, "name", None) or str(k)
            name2sem[nm] = h
        drain_inst.ins.sync_info = mybir.SyncInfo(
            on_wait=[waits[0]], on_update=[]
        )
        for w in waits[1:]:
            h = name2sem.get(w.ant_name)
            assert h is not None, (w.ant_name, list(name2sem))
            n = nc.sync.nop(nofuse=True)
            n.wait_op(h, w.wait_value, "sem-ge")
    nc.all_engine_barrier()
    popped = nc._tile_sem_poison_stack.pop()
    assert popped is self._sem_poison
    nc.clear_and_free_semaphores(list(self.sems.allocated().values()))
    nc.all_engine_barrier()


tile.TileContext._drain_and_barrier = _patched_drain_and_barrier
bass_utils.upload_artifacts = lambda tmpdir: tmpdir

_legalize_counter = [0]


def legalize_waits(nc):
    n_split = 0
    for f in nc.m.functions:
        for bb in f.blocks:
            insts = bb.instructions
            if not any(
                i.sync_info is not None and len(i.sync_info.on_wait) > 1
                for i in insts
            ):
                continue
            new_list = []
            for ins in insts:
                si = ins.sync_info
                if si is not None and len(si.on_wait) > 1:
                    waits = list(si.on_wait)
                    for w in waits[:-1]:
                        _legalize_counter[0] += 1
                        nop = mybir.InstNoOp(
                            name=f"lw_nop_{_legalize_counter[0]}",
                            ins=[], outs=[],
                        )
                        nop.engine = ins.engine
                        nop.sync_info = mybir.SyncInfo(
                            on_wait=[w], on_update=[]
                        )
                        new_list.append(nop)
                        n_split += 1
                    ins.sync_info = mybir.SyncInfo(
                        on_wait=[waits[-1]], on_update=list(si.on_update)
                    )
                new_list.append(ins)
            bb.instructions = new_list
    return n_split


# ─────────────────────────────────────────────────────────────────────
# Kernel build
# ─────────────────────────────────────────────────────────────────────

F32 = mybir.dt.float32
BF16 = mybir.dt.bfloat16

DIM = 1024
HD = 64
B = 2
N = 2048
BN = B * N
SCALE = HD ** -0.5
N_CORES = 8
KT = DIM // 128
MC_B = N // 512   # 4 (512-query units per batch)
NT_B = N // 128   # 16 (128-key tiles per batch)
NSLOT = 144       # 7 full units x16 + 2 half units x16


def _build_nc():
    mm_dt = BF16
    nc = bass.Bass("TRN2", target_bir_lowering=False, debug=False,
                   num_devices=N_CORES)
    # host-packed activation pieces: [128p, KT, tok] so each partition's
    # DMA line is one contiguous DRAM read (kt-major inside a piece)
    xcs = [nc.dram_tensor(f"xc{mc}", [128, KT, 512], mm_dt,
                          kind="ExternalInput") for mc in range(MC_B)]
    xb1 = nc.dram_tensor("xb1", [128, KT, N], mm_dt, kind="ExternalInput")
    # weights as [p, kt*m] so the DMA moves contiguous 2KB lines
    wq = nc.dram_tensor("wq", [128, DIM], mm_dt, kind="ExternalInput")
    wk = nc.dram_tensor("wk", [128, DIM], mm_dt, kind="ExternalInput")
    wv = nc.dram_tensor("wv", [128, DIM], mm_dt, kind="ExternalInput")
    wp = nc.dram_tensor("wp", [128, DIM], mm_dt, kind="ExternalInput")
    out = nc.dram_tensor("out", [BN, DIM], BF16, kind="ExternalOutput")
    out_t = out.ap().rearrange("(c p) d -> p c d", p=128)

    with tile.TileContext(nc) as tc:
        with (
            tc.tile_pool(name="xp", bufs=1) as xp,
            tc.tile_pool(name="wpool", bufs=1) as wpool,
            tc.tile_pool(name="qk", bufs=1) as qk,
            tc.tile_pool(name="pt", bufs=8) as ptp,
            tc.tile_pool(name="small", bufs=2) as sp,
            tc.tile_pool(name="osb", bufs=2) as osbp,
            tc.tile_pool(name="ostage", bufs=3) as osp,
            tc.tile_pool(name="ps_s", bufs=2, space="PSUM") as ps_s,
            tc.tile_pool(name="ps_big", bufs=2, space="PSUM") as ps_big,
            tc.tile_pool(name="ps_o", bufs=2, space="PSUM") as ps_o,
        ):
            # ---- engine warmups (run during the DMA fill) ----
            # tiny exp preloads the ACT spline table (~2.7us otherwise
            # paid inside the first pipeline slot)
            we_in = sp.tile([128, 64], F32, tag="warm_i", bufs=1)
            we_out = sp.tile([128, 64], BF16, tag="warm_o", bufs=1)
            nc.vector.memset(we_in[:], 0.0)
            nc.scalar.activation(we_out[:], we_in[:],
                                 mybir.ActivationFunctionType.Exp,
                                 scale=SCALE)
            # zero-matmul burst keeps the PE busy so the HAM clock gate
            # opens (1.2 -> 2.4 GHz) before the real QKV chains start
            wdum = sp.tile([128, 128], mm_dt, tag="warm_w", bufs=1)
            nc.vector.memset(wdum[:], 0.0)
            warm_ps = ps_big.tile([128, 512], F32, tag="big",
                                  name="warm_ps")
            for i in range(24):
                nc.tensor.matmul(warm_ps[:, 0:128], wdum[:], wdum[:],
                                 start=True, stop=True)

            # ---- input DMAs ----
            # weights on the gpsimd ring; xT pieces split between the
            # sync and scalar (ACT is idle pre-exp) rings. wp isn't
            # needed until the first projection, so it goes last.
            wq_s = wpool.tile([128, KT, 128], mm_dt, tag="wq")
            wk_s = wpool.tile([128, KT, 128], mm_dt, tag="wk")
            wv_s = wpool.tile([128, KT, 128], mm_dt, tag="wv")
            wp_s = wpool.tile([128, DIM], mm_dt, tag="wp")
            for w_d, w_s in ((wq, wq_s), (wk, wk_s), (wv, wv_s)):
                nc.gpsimd.dma_start(
                    out=w_s[:],
                    in_=w_d.ap().rearrange("p (kt m) -> p kt m", kt=KT),
                )
            nc.gpsimd.dma_start(out=wp_s[:], in_=wp.ap())

            xT_s = xp.tile([128, KT, BN], mm_dt, tag="xT")
            # chunk 0 per k-tile, alternating rings, so the first QT/KT
            # chains overlap the arrival of their own operands
            for kt in range(KT):
                eng = nc.sync if kt % 2 == 0 else nc.scalar
                eng.dma_start(
                    out=xT_s[:, kt:kt + 1, 0:512],
                    in_=xcs[0].ap()[:, kt:kt + 1, :],
                )
            for mc in range(1, MC_B):
                nc.sync.dma_start(
                    out=xT_s[:, :, ts(mc, 512)],
                    in_=xcs[mc].ap(),
                )
            # batch 1 in two halves so its V/KT chains can start after
            # the first half lands
            nc.sync.dma_start(out=xT_s[:, :, N:N + 1024],
                              in_=xb1.ap()[:, :, 0:1024])
            nc.sync.dma_start(out=xT_s[:, :, N + 1024:BN],
                              in_=xb1.ap()[:, :, 1024:N])

            # ---- QKV work items ----
            QT_s = qk.tile([128, BN], mm_dt, tag="QT")
            KT_s = qk.tile([128, BN], mm_dt, tag="KT")
            V_s = qk.tile([128, 32, 2, 72], mm_dt, tag="V")
            nc.vector.memset(V_s[:, :, :, 64], 1.0)

            def make_chain(w_s, dst, mc, nsub=2):
                """QT/KT chunk as nsub sub-items accumulating into one
                ps_big tile; released by the copy on the last sub."""
                st = {}
                kts = [range(i * KT // nsub, (i + 1) * KT // nsub)
                       for i in range(nsub)]

                def mk(i):
                    def f():
                        if i == 0:
                            st["ps"] = ps_big.tile(
                                [128, 512], F32, tag="big",
                                name=f"qk_ps_{mc}",
                            )
                        for kt in kts[i]:
                            nc.tensor.matmul(
                                st["ps"][:], w_s[:, kt, :],
                                xT_s[:, kt, ts(mc, 512)],
                                start=(kt == 0), stop=(kt == KT - 1),
                            )
                        if i == nsub - 1:
                            nc.vector.tensor_copy(dst[:, ts(mc, 512)],
                                                  st["ps"][:])
                    return f

                return [mk(i) for i in range(nsub)]

            def item_v(mt):
                def f():
                    ps = ps_big.tile([128, 2, 64], F32, tag="big",
                                     name="v_ps")
                    for kt in range(KT):
                        nc.tensor.matmul(
                            ps[:], xT_s[:, kt, ts(mt, 128)],
                            wv_s[:, kt, :],
                            start=(kt == 0), stop=(kt == KT - 1),
                        )
                    nc.vector.tensor_copy(V_s[:, mt, :, 0:64], ps[:])
                return f

            # last 512-query unit split into two 256-query halves so its
            # norm+proj chain overlaps the second half instead of
            # serializing into the kernel tail
            units = []
            for b in range(B):
                for mc in range(MC_B):
                    u = b * MC_B + mc
                    if u < 7:
                        units.append((b, u * 512, 512))
                    else:
                        units.append((b, u * 512, 256))
                        units.append((b, u * 512 + 256, 256))

            # ---- drip schedule ----
            # EDF with per-slot budgets.  A chunk's two halves go in
            # adjacent slots: half-A last in its slot (with no V item
            # there) and half-B first in the next, so the open ps_big
            # tile sees at most one other allocation (the slot's proj/R)
            # while live — safe under the 2-buffer rotation.
            items = []  # (deadline, kind, payload)
            qt_chains = {mc: make_chain(wq_s, QT_s, mc)
                         for mc in range(2 * MC_B)}
            kt_chains = {mc: make_chain(wk_s, KT_s, mc)
                         for mc in range(1, 2 * MC_B)}
            # chunk 0 of Q^T and K^T interleave across the two pool
            # buffers so both consume each xc0 k-tile piece as it lands
            qt0 = make_chain(wq_s, QT_s, 0, nsub=KT)
            kt0 = make_chain(wk_s, KT_s, 0, nsub=KT)
            pre_items = [f for pair in zip(qt0, kt0) for f in pair] \
                + [item_v(0)]
            for j in (1, 2, 3):           # b=0 key chunks
                items.append((max(4 * j - 2, 1), "c", kt_chains[j]))
            for mt in range(1, 16):       # b=0 value tiles
                items.append((min(mt + 1, 15), "v", item_v(mt)))
            for u in range(1, 8):         # query chunks (4-slot margin)
                items.append((16 * u - 4, "c", qt_chains[u]))
            for j in (0, 1, 2, 3):        # b=1 key chunks
                items.append((60 + 4 * j, "c", kt_chains[4 + j]))
            for j in range(16):           # b=1 value tiles (consumed by
                # the AV pop at slot 67+j under the lag-3 AV discipline)
                items.append((64 + j, "v", item_v(16 + j)))
            items.sort(key=lambda it: it[0])

            slot_w = []
            for _, _, w_ in units:
                slot_w += [w_] * NT_B

            def slot_budget(s):
                if s < 16:
                    return 3100       # unit 0: no pending, DMA-paced
                w_ = slot_w[s]
                slot_len = 2674 if w_ == 512 else 1728
                pend = 512 if s % 16 in (6, 8, 9, 10, 11, 12, 13, 14, 15) \
                    else 0
                return max(slot_len - 3 * w_ - pend + 150, 0)

            # placement rules: a chunk takes a dedicated (otherwise
            # empty) slot pair and accepts the ~+380ns stretch; V items
            # fill remaining capacity with a little slack over budget.
            # Deadline forcing is the backstop for both.
            drip = [[] for _ in range(NSLOT)]
            no_v = [False] * NSLOT
            used = [0] * NSLOT
            ptr = 0
            for s in range(NSLOT):
                while ptr < len(items):
                    dl, kind, payload = items[ptr]
                    if dl - s > 22:
                        break         # too early — spread toward deadline
                    if kind == "v":
                        cyc = 1024
                        if no_v[s]:
                            if dl > s:
                                break
                            # forced into a half-A slot: insert before
                            # the open chunk's half-A (the previous
                            # chunk, if any, is already closed)
                            drip[s].insert(len(drip[s]) - 1, payload)
                            used[s] += cyc
                            ptr += 1
                            continue
                        if dl > s and used[s] + cyc > slot_budget(s) + 350:
                            break
                        drip[s].append(payload)
                        used[s] += cyc
                    else:
                        # chunk halves: A here (and no V after it),
                        # B first in the next slot
                        if s + 1 >= NSLOT:
                            raise AssertionError("chunk spills past end")
                        if dl > s and (used[s] > 0 or used[s + 1] > 0):
                            break
                        drip[s].append(payload[0])
                        used[s] += 2048
                        drip[s + 1].insert(0, payload[1])
                        used[s + 1] += 2048
                        no_v[s] = True
                        ptr += 1
                        break  # at most one half-A per slot
                    ptr += 1
            assert ptr == len(items), (ptr, len(items))

            # ---- attention ----
            # ones64: K=64 stationary for the per-head normalization
            # broadcast (partition bases must be 32-aligned, so the den
            # rows live at {0, 32}); row 0 -> out cols 0-63 (head 0),
            # row 32 -> out cols 64-127 (head 1), rest zero
            ones64_b = sp.tile([64, 128], mm_dt, tag="ones64", bufs=1)
            nc.vector.memset(ones64_b[:], 0.0)
            nc.vector.memset(ones64_b[0:1, 0:64], 1.0)
            nc.vector.memset(ones64_b[32:33, 64:128], 1.0)

            def norm_and_proj_stages(q_off, w, den_s, o_list, tail=False):
                """Stages tagged (slot, position): run at that nt of the
                NEXT unit, 'pre' before / 'post' after the slot's AV
                pop.  Under the lag-3 AV discipline the previous unit's
                last AV (av15) pops at slot 2 and this unit's first AV
                (av0, writing BOTH o banks) at slot 3 — so the h0 copy
                runs post-AV at slot 2 and the h1 copy pre-AV at slot
                3."""
                state = {}

                def mk_copy(h):
                    def f():
                        if h == 0:
                            state["o_sb"] = osbp.tile(
                                [128, 512], F32, tag="osb", name="o_sb")
                        if tail:
                            nc.scalar.copy(state["o_sb"][ts(h, 64), 0:w],
                                           o_list[h][0:64, 0:w])
                        else:
                            nc.vector.tensor_copy(
                                state["o_sb"][ts(h, 64), 0:w],
                                o_list[h][0:64, 0:w])
                        nc.vector.tensor_copy(
                            den_s[32 * h:32 * h + 1, 0:w],
                            o_list[h][64:65, 0:w])
                    return f

                def mk_recip(half):
                    # two half-width calls so other DVE work (the pool
                    # release copies) can interleave; the custom-DVE
                    # reciprocal_approx_* ops don't compile on this
                    # walrus build ("ISA wrong length")
                    def f():
                        if half == 0:
                            state["r"] = sp.tile([64, 512], F32,
                                                 tag="rall", name="r_all")
                        sl = slice(half * (w // 2), (half + 1) * (w // 2))
                        nc.vector.reciprocal(state["r"][:, sl],
                                             den_s[:, sl])
                    return f

                def st_rmat():
                    rb = sp.tile([64, 512], mm_dt, tag="rb", name="r_bf")
                    nc.vector.tensor_copy(rb[:, 0:w], state["r"][:, 0:w])
                    R_ps = ps_big.tile([128, 512], F32, tag="big",
                                       name="R_ps")
                    nc.tensor.matmul(
                        R_ps[:, 0:w], ones64_b[:], rb[:, 0:w],
                        start=True, stop=True,
                    )
                    state["R"] = R_ps

                def st_mul():
                    AT = sp.tile([128, 512], mm_dt, tag="AT", name="AT_s")
                    nc.vector.tensor_mul(AT[:, 0:w], state["o_sb"][:, 0:w],
                                         state["R"][:, 0:w])
                    state["AT"] = AT

                def mk_proj(mt, cc):
                    def f():
                        if "out" not in state:
                            state["out"] = osp.tile(
                                [128, 4, DIM], BF16, tag="out",
                                name="out_stage",
                            )
                        p_ps = ps_big.tile([128, 512], F32, tag="big",
                                           name="p_ps")
                        nc.tensor.matmul(
                            p_ps[:], state["AT"][:, ts(mt, 128)],
                            wp_s[:, ts(cc, 512)],
                            start=True, stop=True,
                        )
                        if tail:
                            nc.scalar.copy(
                                state["out"][:, mt, ts(cc, 512)], p_ps[:])
                            # small pieces on both rings so the final
                            # DMA drain is short
                            for q4 in range(2 * cc, 2 * cc + 2):
                                eng = nc.sync if q4 % 2 == 0 \
                                    else nc.gpsimd
                                eng.dma_start(
                                    out=out_t[:, q_off // 128 + mt,
                                              ts(q4, 256)],
                                    in_=state["out"][:, mt, ts(q4, 256)],
                                )
                        else:
                            nc.vector.tensor_copy(
                                state["out"][:, mt, ts(cc, 512)], p_ps[:])
                            if cc == 1:
                                eng = nc.sync if mt % 2 == 0 \
                                    else nc.gpsimd
                                eng.dma_start(
                                    out=out_t[:, q_off // 128 + mt, :],
                                    in_=state["out"][:, mt, :],
                                )
                    return f

                return (
                    [(2, "post", mk_copy(0)), (3, "pre", mk_copy(1)),
                     (4, "post", mk_recip(0)), (5, "post", mk_recip(1)),
                     (6, "post", st_rmat), (7, "post", st_mul)]
                    + [(8 + k, "post", mk_proj(k // 2, k % 2))
                       for k in range(2 * (w // 128))]
                )

            pending = []
            av_q = []  # AV emission runs 3 slots behind scores/exp so
            # the in-order PE queue never stalls waiting on the ACT exp

            def mk_av(o_l, bb, ntt, pt, ww):
                def av():
                    for h in range(2):
                        nc.tensor.matmul(
                            o_l[h][0:65, 0:ww],
                            V_s[:, bb * NT_B + ntt, h, 0:65],
                            pt[:, h, 0:ww],
                            start=(ntt == 0),
                            stop=(ntt == NT_B - 1),
                        )
                return av

            for fn in pre_items:
                fn()

            slot = 0
            for ui, (b, q_off, w) in enumerate(units):
                is_last = ui == len(units) - 1
                # den rows {0, 32}; memset (on the idle gpsimd engine)
                # so the unused rows can't feed inf/nan into the bf16
                # broadcast matmul
                den_s = sp.tile([64, 512], F32, tag="den")
                nc.gpsimd.memset(den_s[:], 1.0)
                o_list = [ps_o.tile([128, 512], F32, tag="o",
                                    name=f"o_ps_{h}")
                          for h in range(2)]
                for nt in range(NT_B):
                    # scores first: the exp (the pipeline rate limiter)
                    # only ever waits on this pair, never on drip work
                    s_ps = ps_s.tile([128, 2, 512], F32, tag="S")
                    for h in range(2):
                        h_sl = ts(h, 64)
                        nc.tensor.matmul(
                            s_ps[:, h, 0:w],
                            KT_s[h_sl, b * N + nt * 128:
                                 b * N + (nt + 1) * 128],
                            QT_s[h_sl, q_off:q_off + w],
                            start=True, stop=True,
                        )
                    PT_s = ptp.tile([128, 2, 512], mm_dt, tag="PT")
                    nc.scalar.activation(
                        PT_s[:, :, 0:w], s_ps[:, :, 0:w],
                        mybir.ActivationFunctionType.Exp,
                        scale=SCALE,
                    )
                    for sl_, pos, fn in pending:
                        if sl_ == nt and pos == "pre":
                            fn()
                    av_q.append(mk_av(o_list, b, nt, PT_s, w))
                    # in the last unit drain the AV backlog early so the
                    # tail norm chain starts sooner
                    lag = 1 if (is_last and nt >= 13) else 3
                    while len(av_q) > lag:
                        av_q.pop(0)()
                    # drip before the post stages so the pool-release
                    # copies sit early in the DVE queue
                    for fn in drip[slot]:
                        fn()
                    for sl_, pos, fn in pending:
                        if sl_ == nt and pos == "post":
                            fn()
                    slot += 1
                pending = norm_and_proj_stages(
                    q_off, w, den_s, o_list, tail=(q_off + w == BN))
            while av_q:
                av_q.pop(0)()
            for _, _, fn in pending:
                fn()
    legalize_waits(nc)
    return nc


_CACHE = {}


def _get_nc():
    if "nc" not in _CACHE:
        _CACHE["nc"] = _build_nc()
    return _CACHE["nc"]


# ─────────────────────────────────────────────────────────────────────
# Host-side packing
# ─────────────────────────────────────────────────────────────────────

def wpack_test(w):
    # [DIM, 128] -> [128p, KT*128] so each SBUF partition line is one
    # contiguous 2KB DMA read
    return np.ascontiguousarray(
        np.asarray(w, dtype=np.float32)
        .reshape(KT, 128, 128).transpose(1, 0, 2).reshape(128, DIM)
    ).astype(ml_dtypes.bfloat16)


def xpack_test(x):
    """Full x [B, N, DIM] -> dict of contiguous bf16 DMA pieces in
    [128p, KT, tok] layout (per-partition lines are contiguous DRAM)."""
    bf = ml_dtypes.bfloat16
    xT = np.asarray(x, dtype=np.float32).reshape(BN, DIM).T  # [DIM, BN]
    pieces = {}
    for mc in range(MC_B):
        pieces[f"xc{mc}"] = np.ascontiguousarray(
            xT[:, mc * 512:(mc + 1) * 512]
            .reshape(KT, 128, 512).transpose(1, 0, 2)
        ).astype(bf)
    pieces["xb1"] = np.ascontiguousarray(
        xT[:, N:BN].reshape(KT, 128, N).transpose(1, 0, 2)
    ).astype(bf)
    return pieces


def build_in_maps(x, w_qkv, w_proj):
    """Per-core input maps (shared xT pieces, per-core weight slices)."""
    w_qkv = np.asarray(w_qkv, dtype=np.float32)
    w_proj = np.asarray(w_proj, dtype=np.float32)
    xp = xpack_test(x)
    bf = ml_dtypes.bfloat16
    in_maps = []
    for c in range(N_CORES):
        sl = slice(128 * c, 128 * (c + 1))
        in_maps.append({
            **xp,
            "wq": wpack_test(w_qkv[:, sl]),
            "wk": wpack_test(w_qkv[:, DIM + 128 * c:DIM + 128 * (c + 1)]),
            "wv": wpack_test(
                w_qkv[:, 2 * DIM + 128 * c:2 * DIM + 128 * (c + 1)]),
            "wp": np.ascontiguousarray(w_proj[sl, :]).astype(bf),
        })
    return in_maps


def kernel(x, w_qkv, w_proj, b_proj):
    x = np.asarray(x, dtype=np.float32)
    b_proj = np.asarray(b_proj, dtype=np.float32)

    nc = _get_nc()
    in_maps = build_in_maps(x, w_qkv, w_proj)
    res = run_bass_kernel_spmd(nc, in_maps, list(range(N_CORES)),
                               trace=False)
    acc = res.results[0]["out"].astype(np.float32).copy()
    for c in range(1, N_CORES):
        acc += res.results[c]["out"]
    acc += b_proj[None, :]
    return acc.reshape(B, N, DIM)


# revision 51
# speedup vs baseline: 1.0647x; 1.0288x over previous
"""Multi-head self-attention (B=2, N=2048, D=1024, 16 heads) on 8 TRN2
NeuronCores — tensor-parallel over heads (2 heads per core), row-parallel
output projection summed on the host.

Self-contained: takes the FULL inputs of reference.setup_inputs() and
returns the FULL [2, 2048, 1024] fp32 output.

Per-core device kernel (Bass/Tile, bf16 matmuls, fp32 accumulation).
The attention phase is a software pipeline of 144 "slots" (one slot =
one 128-key tile of one 512-query unit); the ACT-engine exp
((312+1024)cyc/1.2GHz ~ 1.11us per slot) is the rate limiter, so the
schedule keeps ACT exp-only and keeps the PE's in-order queue free of
slow-gated work ahead of the score matmuls:

  slot order: score pair (both heads, disjoint PE row groups; emitted
  first so the exp only ever waits on it) -> exp -> previous unit's
  norm stages -> AV pair (lag 3, ones-column emits softmax
  denominators for free) -> drip items -> PE norm/proj stages.

  QKV/proj/V/R share one 2-buffer PSUM pool (ps_big) so every
  MM->copy->MM sequence has a full slot of slack; QT/KT chunks drip as
  two 4-ktile halves in adjacent slots (a chunk tile may only see ONE
  other pool allocation while open, which the scheduler guarantees by
  keeping V items out of half-A slots and emitting half-B first in its
  slot).  Items are deadline-scheduled (EDF) with per-slot budgets.

  norm stages of unit u pop at fixed slots of unit u+1: o_sb copy h0 @2
  (after the previous unit's last AV retires the o banks), h1 @3
  (before this unit's first AV pop), reciprocal halves @4,5, 1/den
  broadcast matmul @6, o*R mul @7, eight single-MM projections @8..15.

  front: weights on the gpsimd DMA ring; host-packed contiguous xT
  pieces on the sync+scalar rings (chunk 0 per-ktile, alternating);
  Q^T/K^T chunk-0 chains interleave on the two pool buffers so both
  consume each piece as it lands; a zero-matmul burst warms the PE
  clock (HAM) and a tiny exp preloads the ACT table during the fill.

  tail: last 512-query unit split into two 256-query halves so its
  norm+proj chain overlaps the second half; AV lag drops to 1 for the
  last slots; final output DMAs go out in 64KB pieces on alternating
  sync/gpsimd rings so the post-kernel drain is short.
"""

import sys

sys.path.insert(0, "/opt/trn_rl_repo")

import numpy as np
import ml_dtypes

import concourse.mybir as mybir
import concourse.tile as tile
import concourse.bass as bass
from concourse.bass import ts
from concourse import bass_utils
from concourse.bass_utils import run_bass_kernel_spmd

# ─────────────────────────────────────────────────────────────────────
# Environment patches (this walrus build allows only ONE sem wait per
# instruction; Tile emits several — split them into single-wait nops).
# ─────────────────────────────────────────────────────────────────────


def _patched_drain_and_barrier(self, tick_clock, wait_clock):
    from concourse.tile import ScopedClock

    nc = self.nc
    drain_inst = nc.sync.drain()
    wait_clock.add_sem_waits(
        drain_inst.ins, ScopedClock({None: tick_clock.global_clock})
    )
    waits = list(drain_inst.ins.sync_info.on_wait)
    if len(waits) > 1:
        name2sem = {}
        for k, h in self.sems.allocated().items():
            nm = getattr(h, "name", None) or str(k)
            name2sem[nm] = h
        drain_inst.ins.sync_info = mybir.SyncInfo(
            on_wait=[waits[0]], on_update=[]
        )
        for w in waits[1:]:
            h = name2sem.get(w.ant_name)
            assert h is not None, (w.ant_name, list(name2sem))
            n = nc.sync.nop(nofuse=True)
            n.wait_op(h, w.wait_value, "sem-ge")
    nc.all_engine_barrier()
    popped = nc._tile_sem_poison_stack.pop()
    assert popped is self._sem_poison
    nc.clear_and_free_semaphores(list(self.sems.allocated().values()))
    nc.all_engine_barrier()


tile.TileContext._drain_and_barrier = _patched_drain_and_barrier
bass_utils.upload_artifacts = lambda tmpdir: tmpdir

_legalize_counter = [0]


def legalize_waits(nc):
    n_split = 0
    for f in nc.m.functions:
        for bb in f.blocks:
            insts = bb.instructions
            if not any(
                i.sync_info is not None and len(i.sync_info.on_wait) > 1
                for i in insts
            ):
                continue
            new_list = []
            for ins in insts:
                si = ins.sync_info
                if si is not None and len(si.on_wait) > 1:
                    waits = list(si.on_wait)
                    for w in waits[:-1]:
                        _legalize_counter[0] += 1
                        nop = mybir.InstNoOp(
                            name=f"lw_nop_{_legalize_counter[0]}",
                            ins=[], outs=[],
                        )
                        nop.engine = ins.engine
                        nop.sync_info = mybir.SyncInfo(
                            on_wait=[w], on_update=[]
                        )
                        new_list.append(nop)
                        n_split += 1
                    ins.sync_info = mybir.SyncInfo(
                        on_wait=[waits[-1]], on_update=list(si.on_update)
                    )
                new_list.append(ins)
            bb.instructions = new_list
    return n_split


# ─────────────────────────────────────────────────────────────────────
# Kernel build
# ─────────────────────────────────────────────────────────────────────

F32 = mybir.dt.float32
BF16 = mybir.dt.bfloat16

DIM = 1024
HD = 64
B = 2
N = 2048
BN = B * N
SCALE = HD ** -0.5
N_CORES = 8
KT = DIM // 128
MC_B = N // 512   # 4 (512-query units per batch)
NT_B = N // 128   # 16 (128-key tiles per batch)
NSLOT = 128       # 8 units x 16 key-tile slots


def _build_nc():
    mm_dt = BF16
    nc = bass.Bass("TRN2", target_bir_lowering=False, debug=False,
                   num_devices=N_CORES)
    # host-packed activation pieces: [128p, KT, tok] so each partition's
    # DMA line is one contiguous DRAM read (kt-major inside a piece)
    xcs = [nc.dram_tensor(f"xc{mc}", [128, KT, 512], mm_dt,
                          kind="ExternalInput") for mc in range(MC_B)]
    xb1 = nc.dram_tensor("xb1", [128, KT, N], mm_dt, kind="ExternalInput")
    # weights as [p, kt*m] so the DMA moves contiguous 2KB lines
    wq = nc.dram_tensor("wq", [128, DIM], mm_dt, kind="ExternalInput")
    wk = nc.dram_tensor("wk", [128, DIM], mm_dt, kind="ExternalInput")
    wv = nc.dram_tensor("wv", [128, DIM], mm_dt, kind="ExternalInput")
    wp = nc.dram_tensor("wp", [128, DIM], mm_dt, kind="ExternalInput")
    out = nc.dram_tensor("out", [BN, DIM], BF16, kind="ExternalOutput")
    out_t = out.ap().rearrange("(c p) d -> p c d", p=128)

    with tile.TileContext(nc) as tc:
        with (
            tc.tile_pool(name="xp", bufs=1) as xp,
            tc.tile_pool(name="wpool", bufs=1) as wpool,
            tc.tile_pool(name="qk", bufs=1) as qk,
            tc.tile_pool(name="pt", bufs=8) as ptp,
            tc.tile_pool(name="small", bufs=2) as sp,
            tc.tile_pool(name="osb", bufs=2) as osbp,
            tc.tile_pool(name="ostage", bufs=3) as osp,
            tc.tile_pool(name="ps_s", bufs=2, space="PSUM") as ps_s,
            tc.tile_pool(name="ps_big", bufs=2, space="PSUM") as ps_big,
            tc.tile_pool(name="ps_o", bufs=2, space="PSUM") as ps_o,
        ):
            # ---- engine warmups (run during the DMA fill) ----
            # tiny exp preloads the ACT spline table (~2.7us otherwise
            # paid inside the first pipeline slot)
            we_in = sp.tile([128, 64], F32, tag="warm_i", bufs=1)
            we_out = sp.tile([128, 64], BF16, tag="warm_o", bufs=1)
            nc.vector.memset(we_in[:], 0.0)
            nc.scalar.activation(we_out[:], we_in[:],
                                 mybir.ActivationFunctionType.Exp,
                                 scale=SCALE)
            # zero-matmul burst keeps the PE busy so the HAM clock gate
            # opens (1.2 -> 2.4 GHz) before the real QKV chains start
            wdum = sp.tile([128, 128], mm_dt, tag="warm_w", bufs=1)
            nc.vector.memset(wdum[:], 0.0)
            warm_ps = ps_big.tile([128, 512], F32, tag="big",
                                  name="warm_ps")
            for i in range(24):
                nc.tensor.matmul(warm_ps[:, 0:128], wdum[:], wdum[:],
                                 start=True, stop=True)

            # ---- input DMAs ----
            # weights on the gpsimd ring; xT pieces split between the
            # sync and scalar (ACT is idle pre-exp) rings. wp isn't
            # needed until the first projection, so it goes last.
            wq_s = wpool.tile([128, KT, 128], mm_dt, tag="wq")
            wk_s = wpool.tile([128, KT, 128], mm_dt, tag="wk")
            wv_s = wpool.tile([128, KT, 128], mm_dt, tag="wv")
            wp_s = wpool.tile([128, DIM], mm_dt, tag="wp")
            for w_d, w_s in ((wq, wq_s), (wk, wk_s), (wv, wv_s)):
                nc.gpsimd.dma_start(
                    out=w_s[:],
                    in_=w_d.ap().rearrange("p (kt m) -> p kt m", kt=KT),
                )
            nc.gpsimd.dma_start(out=wp_s[:], in_=wp.ap())

            xT_s = xp.tile([128, KT, BN], mm_dt, tag="xT")
            # chunk 0 per k-tile, alternating rings, so the first QT/KT
            # chains overlap the arrival of their own operands
            for kt in range(KT):
                eng = nc.sync if kt % 2 == 0 else nc.scalar
                eng.dma_start(
                    out=xT_s[:, kt:kt + 1, 0:512],
                    in_=xcs[0].ap()[:, kt:kt + 1, :],
                )
            for mc in range(1, MC_B):
                nc.sync.dma_start(
                    out=xT_s[:, :, ts(mc, 512)],
                    in_=xcs[mc].ap(),
                )
            # batch 1 in two halves so its V/KT chains can start after
            # the first half lands
            nc.sync.dma_start(out=xT_s[:, :, N:N + 1024],
                              in_=xb1.ap()[:, :, 0:1024])
            nc.sync.dma_start(out=xT_s[:, :, N + 1024:BN],
                              in_=xb1.ap()[:, :, 1024:N])

            # ---- QKV work items ----
            QT_s = qk.tile([128, BN], mm_dt, tag="QT")
            KT_s = qk.tile([128, BN], mm_dt, tag="KT")
            V_s = qk.tile([128, 32, 2, 72], mm_dt, tag="V")
            nc.vector.memset(V_s[:, :, :, 64], 1.0)

            def make_chain(w_s, dst, mc, nsub=2):
                """QT/KT chunk as nsub sub-items accumulating into one
                ps_big tile; released by the copy on the last sub."""
                st = {}
                kts = [range(i * KT // nsub, (i + 1) * KT // nsub)
                       for i in range(nsub)]

                def mk(i):
                    def f():
                        if i == 0:
                            st["ps"] = ps_big.tile(
                                [128, 512], F32, tag="big",
                                name=f"qk_ps_{mc}",
                            )
                        for kt in kts[i]:
                            nc.tensor.matmul(
                                st["ps"][:], w_s[:, kt, :],
                                xT_s[:, kt, ts(mc, 512)],
                                start=(kt == 0), stop=(kt == KT - 1),
                            )
                        if i == nsub - 1:
                            nc.vector.tensor_copy(dst[:, ts(mc, 512)],
                                                  st["ps"][:])
                    return f

                return [mk(i) for i in range(nsub)]

            def item_v(mt):
                def f():
                    ps = ps_big.tile([128, 2, 64], F32, tag="big",
                                     name="v_ps")
                    for kt in range(KT):
                        nc.tensor.matmul(
                            ps[:], xT_s[:, kt, ts(mt, 128)],
                            wv_s[:, kt, :],
                            start=(kt == 0), stop=(kt == KT - 1),
                        )
                    nc.vector.tensor_copy(V_s[:, mt, :, 0:64], ps[:])
                return f

            # 8 full 512-query units; the last unit's norm+proj chain
            # runs in the tail with the AV backlog drained early (the
            # 256-query split variant lost more in DVE-bound half-slots
            # than it saved in tail overlap)
            units = [(b, (b * MC_B + mc) * 512, 512)
                     for b in range(B) for mc in range(MC_B)]

            # ---- drip schedule ----
            # EDF with per-slot budgets.  A chunk's two halves go in
            # adjacent slots: half-A last in its slot (with no V item
            # there) and half-B first in the next, so the open ps_big
            # tile sees at most one other allocation (the slot's proj/R)
            # while live — safe under the 2-buffer rotation.
            items = []  # (deadline, kind, payload)
            qt_chains = {mc: make_chain(wq_s, QT_s, mc)
                         for mc in range(2 * MC_B)}
            kt_chains = {mc: make_chain(wk_s, KT_s, mc)
                         for mc in range(1, 2 * MC_B)}
            # chunk 0 of Q^T and K^T interleave across the two pool
            # buffers so both consume each xc0 k-tile piece as it lands
            qt0 = make_chain(wq_s, QT_s, 0, nsub=KT)
            kt0 = make_chain(wk_s, KT_s, 0, nsub=KT)
            pre_items = [f for pair in zip(qt0, kt0) for f in pair] \
                + [item_v(0)]
            for j in (1, 2, 3):           # b=0 key chunks
                items.append((max(4 * j - 2, 1), "c", kt_chains[j]))
            for mt in range(1, 16):       # b=0 value tiles
                items.append((min(mt + 1, 15), "v", item_v(mt)))
            for u in range(1, 8):         # query chunks (4-slot margin)
                items.append((16 * u - 4, "c", qt_chains[u]))
            for j in (0, 1, 2, 3):        # b=1 key chunks
                items.append((60 + 4 * j, "c", kt_chains[4 + j]))
            for j in range(16):           # b=1 value tiles (consumed by
                # the AV pop at slot 67+j under the lag-3 AV discipline)
                items.append((64 + j, "v", item_v(16 + j)))
            items.sort(key=lambda it: it[0])

            slot_w = []
            for _, _, w_ in units:
                slot_w += [w_] * NT_B

            def slot_budget(s):
                if s < 16:
                    return 3100       # unit 0: no pending, DMA-paced
                pend = 512 if s % 16 in (6, 8, 9, 10, 11, 12, 13, 14, 15) \
                    else 0
                return max(2674 - 1536 - pend + 150, 0)

            # placement rules: a chunk takes a dedicated (otherwise
            # empty) slot pair and accepts the ~+380ns stretch; V items
            # fill remaining capacity with a little slack over budget.
            # Deadline forcing is the backstop for both.
            drip = [[] for _ in range(NSLOT)]
            no_v = [False] * NSLOT
            used = [0] * NSLOT
            ptr = 0
            for s in range(NSLOT):
                while ptr < len(items):
                    dl, kind, payload = items[ptr]
                    if dl - s > 22:
                        break         # too early — spread toward deadline
                    if kind == "v":
                        cyc = 1024
                        if no_v[s]:
                            if dl > s:
                                break
                            # forced into a half-A slot: insert before
                            # the open chunk's half-A (the previous
                            # chunk, if any, is already closed)
                            drip[s].insert(len(drip[s]) - 1, payload)
                            used[s] += cyc
                            ptr += 1
                            continue
                        if dl > s and used[s] + cyc > slot_budget(s) + 350:
                            break
                        drip[s].append(payload)
                        used[s] += cyc
                    else:
                        # chunk halves: A here (and no V after it),
                        # B first in the next slot
                        if s + 1 >= NSLOT:
                            raise AssertionError("chunk spills past end")
                        if dl > s and (used[s] > 0 or used[s + 1] > 0):
                            break
                        drip[s].append(payload[0])
                        used[s] += 2048
                        drip[s + 1].insert(0, payload[1])
                        used[s + 1] += 2048
                        no_v[s] = True
                        ptr += 1
                        break  # at most one half-A per slot
                    ptr += 1
            assert ptr == len(items), (ptr, len(items))

            # ---- attention ----
            # ones64: K=64 stationary for the per-head normalization
            # broadcast (partition bases must be 32-aligned, so the den
            # rows live at {0, 32}); row 0 -> out cols 0-63 (head 0),
            # row 32 -> out cols 64-127 (head 1), rest zero
            ones64_b = sp.tile([64, 128], mm_dt, tag="ones64", bufs=1)
            nc.vector.memset(ones64_b[:], 0.0)
            nc.vector.memset(ones64_b[0:1, 0:64], 1.0)
            nc.vector.memset(ones64_b[32:33, 64:128], 1.0)

            def norm_and_proj_stages(q_off, w, den_s, o_list, tail=False):
                """Stages tagged (slot, position): run at that nt of the
                NEXT unit, 'pre' before / 'post' after the slot's AV
                pop.  Under the lag-3 AV discipline the previous unit's
                last AV (av15) pops at slot 2 and this unit's first AV
                (av0, writing BOTH o banks) at slot 3 — so the h0 copy
                runs post-AV at slot 2 and the h1 copy pre-AV at slot
                3."""
                state = {}

                def mk_copy(h):
                    def f():
                        if h == 0:
                            state["o_sb"] = osbp.tile(
                                [128, 512], F32, tag="osb", name="o_sb")
                        if tail:
                            nc.scalar.copy(state["o_sb"][ts(h, 64), 0:w],
                                           o_list[h][0:64, 0:w])
                        else:
                            nc.vector.tensor_copy(
                                state["o_sb"][ts(h, 64), 0:w],
                                o_list[h][0:64, 0:w])
                        nc.vector.tensor_copy(
                            den_s[32 * h:32 * h + 1, 0:w],
                            o_list[h][64:65, 0:w])
                    return f

                def mk_recip(half):
                    # two half-width calls so other DVE work (the pool
                    # release copies) can interleave; the custom-DVE
                    # reciprocal_approx_* ops don't compile on this
                    # walrus build ("ISA wrong length")
                    def f():
                        if half == 0:
                            state["r"] = sp.tile([64, 512], F32,
                                                 tag="rall", name="r_all")
                        sl = slice(half * (w // 2), (half + 1) * (w // 2))
                        nc.vector.reciprocal(state["r"][:, sl],
                                             den_s[:, sl])
                    return f

                def st_rmat():
                    rb = sp.tile([64, 512], mm_dt, tag="rb", name="r_bf")
                    nc.vector.tensor_copy(rb[:, 0:w], state["r"][:, 0:w])
                    R_ps = ps_big.tile([128, 512], F32, tag="big",
                                       name="R_ps")
                    nc.tensor.matmul(
                        R_ps[:, 0:w], ones64_b[:], rb[:, 0:w],
                        start=True, stop=True,
                    )
                    state["R"] = R_ps

                def st_mul():
                    AT = sp.tile([128, 512], mm_dt, tag="AT", name="AT_s")
                    nc.vector.tensor_mul(AT[:, 0:w], state["o_sb"][:, 0:w],
                                         state["R"][:, 0:w])
                    state["AT"] = AT

                def mk_proj(mt, cc):
                    def f():
                        if "out" not in state:
                            state["out"] = osp.tile(
                                [128, 4, DIM], BF16, tag="out",
                                name="out_stage",
                            )
                        p_ps = ps_big.tile([128, 512], F32, tag="big",
                                           name="p_ps")
                        nc.tensor.matmul(
                            p_ps[:], state["AT"][:, ts(mt, 128)],
                            wp_s[:, ts(cc, 512)],
                            start=True, stop=True,
                        )
                        if tail:
                            nc.scalar.copy(
                                state["out"][:, mt, ts(cc, 512)], p_ps[:])
                            # small pieces on both rings so the final
                            # DMA drain is short
                            for q4 in range(2 * cc, 2 * cc + 2):
                                eng = nc.sync if q4 % 2 == 0 \
                                    else nc.gpsimd
                                eng.dma_start(
                                    out=out_t[:, q_off // 128 + mt,
                                              ts(q4, 256)],
                                    in_=state["out"][:, mt, ts(q4, 256)],
                                )
                        else:
                            nc.vector.tensor_copy(
                                state["out"][:, mt, ts(cc, 512)], p_ps[:])
                            if cc == 1:
                                eng = nc.sync if mt % 2 == 0 \
                                    else nc.gpsimd
                                eng.dma_start(
                                    out=out_t[:, q_off // 128 + mt, :],
                                    in_=state["out"][:, mt, :],
                                )
                    return f

                return (
                    [(2, "post", mk_copy(0)), (3, "pre", mk_copy(1)),
                     (4, "post", mk_recip(0)), (5, "post", mk_recip(1)),
                     (6, "post", st_rmat), (7, "post", st_mul)]
                    + [(8 + k, "post", mk_proj(k // 2, k % 2))
                       for k in range(2 * (w // 128))]
                )

            pending = []
            av_q = []  # AV emission runs 3 slots behind scores/exp so
            # the in-order PE queue never stalls waiting on the ACT exp

            def mk_av(o_l, bb, ntt, pt, ww):
                def av():
                    for h in range(2):
                        nc.tensor.matmul(
                            o_l[h][0:65, 0:ww],
                            V_s[:, bb * NT_B + ntt, h, 0:65],
                            pt[:, h, 0:ww],
                            start=(ntt == 0),
                            stop=(ntt == NT_B - 1),
                        )
                return av

            for fn in pre_items:
                fn()

            slot = 0
            for ui, (b, q_off, w) in enumerate(units):
                is_last = ui == len(units) - 1
                # den rows {0, 32}; memset (on the idle gpsimd engine)
                # so the unused rows can't feed inf/nan into the bf16
                # broadcast matmul
                den_s = sp.tile([64, 512], F32, tag="den")
                nc.gpsimd.memset(den_s[:], 1.0)
                o_list = [ps_o.tile([128, 512], F32, tag="o",
                                    name=f"o_ps_{h}")
                          for h in range(2)]
                for nt in range(NT_B):
                    # scores first: the exp (the pipeline rate limiter)
                    # only ever waits on this pair, never on drip work
                    s_ps = ps_s.tile([128, 2, 512], F32, tag="S")
                    for h in range(2):
                        h_sl = ts(h, 64)
                        nc.tensor.matmul(
                            s_ps[:, h, 0:w],
                            KT_s[h_sl, b * N + nt * 128:
                                 b * N + (nt + 1) * 128],
                            QT_s[h_sl, q_off:q_off + w],
                            start=True, stop=True,
                        )
                    PT_s = ptp.tile([128, 2, 512], mm_dt, tag="PT")
                    nc.scalar.activation(
                        PT_s[:, :, 0:w], s_ps[:, :, 0:w],
                        mybir.ActivationFunctionType.Exp,
                        scale=SCALE,
                    )
                    for sl_, pos, fn in pending:
                        if sl_ == nt and pos == "pre":
                            fn()
                    av_q.append(mk_av(o_list, b, nt, PT_s, w))
                    # in the last unit drain the AV backlog early so the
                    # tail norm chain starts sooner
                    lag = 1 if (is_last and nt >= 13) else 3
                    while len(av_q) > lag:
                        av_q.pop(0)()
                    # drip before the post stages so the pool-release
                    # copies sit early in the DVE queue
                    for fn in drip[slot]:
                        fn()
                    for sl_, pos, fn in pending:
                        if sl_ == nt and pos == "post":
                            fn()
                    slot += 1
                pending = norm_and_proj_stages(
                    q_off, w, den_s, o_list, tail=(q_off + w == BN))
            while av_q:
                av_q.pop(0)()
            for _, _, fn in pending:
                fn()
    legalize_waits(nc)
    return nc


_CACHE = {}


def _get_nc():
    if "nc" not in _CACHE:
        _CACHE["nc"] = _build_nc()
    return _CACHE["nc"]


# ─────────────────────────────────────────────────────────────────────
# Host-side packing
# ─────────────────────────────────────────────────────────────────────

def wpack_test(w):
    # [DIM, 128] -> [128p, KT*128] so each SBUF partition line is one
    # contiguous 2KB DMA read
    return np.ascontiguousarray(
        np.asarray(w, dtype=np.float32)
        .reshape(KT, 128, 128).transpose(1, 0, 2).reshape(128, DIM)
    ).astype(ml_dtypes.bfloat16)


def xpack_test(x):
    """Full x [B, N, DIM] -> dict of contiguous bf16 DMA pieces in
    [128p, KT, tok] layout (per-partition lines are contiguous DRAM)."""
    bf = ml_dtypes.bfloat16
    xT = np.asarray(x, dtype=np.float32).reshape(BN, DIM).T  # [DIM, BN]
    pieces = {}
    for mc in range(MC_B):
        pieces[f"xc{mc}"] = np.ascontiguousarray(
            xT[:, mc * 512:(mc + 1) * 512]
            .reshape(KT, 128, 512).transpose(1, 0, 2)
        ).astype(bf)
    pieces["xb1"] = np.ascontiguousarray(
        xT[:, N:BN].reshape(KT, 128, N).transpose(1, 0, 2)
    ).astype(bf)
    return pieces


def build_in_maps(x, w_qkv, w_proj):
    """Per-core input maps (shared xT pieces, per-core weight slices)."""
    w_qkv = np.asarray(w_qkv, dtype=np.float32)
    w_proj = np.asarray(w_proj, dtype=np.float32)
    xp = xpack_test(x)
    bf = ml_dtypes.bfloat16
    in_maps = []
    for c in range(N_CORES):
        sl = slice(128 * c, 128 * (c + 1))
        in_maps.append({
            **xp,
            "wq": wpack_test(w_qkv[:, sl]),
            "wk": wpack_test(w_qkv[:, DIM + 128 * c:DIM + 128 * (c + 1)]),
            "wv": wpack_test(
                w_qkv[:, 2 * DIM + 128 * c:2 * DIM + 128 * (c + 1)]),
            "wp": np.ascontiguousarray(w_proj[sl, :]).astype(bf),
        })
    return in_maps


def kernel(x, w_qkv, w_proj, b_proj):
    x = np.asarray(x, dtype=np.float32)
    b_proj = np.asarray(b_proj, dtype=np.float32)

    nc = _get_nc()
    in_maps = build_in_maps(x, w_qkv, w_proj)
    res = run_bass_kernel_spmd(nc, in_maps, list(range(N_CORES)),
                               trace=False)
    acc = res.results[0]["out"].astype(np.float32).copy()
    for c in range(1, N_CORES):
        acc += res.results[c]["out"]
    acc += b_proj[None, :]
    return acc.reshape(B, N, DIM)


# revision 53
# speedup vs baseline: 1.1373x; 1.0683x over previous
"""Multi-head self-attention (B=2, N=2048, D=1024, 16 heads) on 8 TRN2
NeuronCores — tensor-parallel over heads (2 heads per core), row-parallel
output projection summed on the host.

Self-contained: takes the FULL inputs of reference.setup_inputs() and
returns the FULL [2, 2048, 1024] fp32 output.

Per-core device kernel (Bass/Tile, bf16 matmuls, fp32 accumulation):
    xT [1024, 4096]  (host-pretransposed activations, chunk-major DMA so
    the first QKV matmul starts ~13us in instead of ~21us)
    Q^T/K^T stacks [128(2 heads x 64), 4096];  V [tok, 2, 64] + ones col
    S^T chunks in PSUM -> one ACT exp per n-tile (scale folded, both
    heads' score matmuls in disjoint PE row groups run concurrently)
    -> P^T bf16;  AV emission runs 2 n-tiles behind scores/exp so the
    in-order PE queue never stalls on the ACT exp
    PV matmul with a ones column emits softmax denominators for free
    normalization: DVE reciprocal (halves for 512-wide units, one
    call for 256-wide ones - DVE recip has a ~1.2us floor) + K=64 bf16
    PE broadcast matmul building both heads' scale rows; the recip/R/
    mul/proj stages of unit u pop at fixed nt slots of unit u+1 (R
    far from the recips so it never blocks the in-order PE queue)
    last 512-query unit split into two 256-query halves so its norm+
    proj chain overlaps the second half instead of the kernel tail;
    tail PSUM->SBUF copies go to the (then idle) ACT engine
    row-parallel proj partial [4096, 1024] bf16 -> DRAM (per-mt DMA),
    host sums the 8 partials in fp32 and adds the bias
"""

import sys
import types

sys.path.insert(0, "/opt/trn_rl_repo")

import numpy as np
import ml_dtypes

import concourse.mybir as mybir
import concourse.tile as tile
import concourse.bass as bass
from concourse.bass import ts
from concourse import bass_utils
from concourse.bass_utils import run_bass_kernel_spmd

# ─────────────────────────────────────────────────────────────────────
# Environment patches (this walrus build allows only ONE sem wait per
# instruction; Tile emits several — split them into single-wait nops).
# ─────────────────────────────────────────────────────────────────────


def _patched_drain_and_barrier(self, tick_clock, wait_clock):
    from concourse.tile import ScopedClock

    nc = self.nc
    drain_inst = nc.sync.drain()
    wait_clock.add_sem_waits(
        drain_inst.ins, ScopedClock({None: tick_clock.global_clock})
    )
    waits = list(drain_inst.ins.sync_info.on_wait)
    if len(waits) > 1:
        name2sem = {}
        for k, h in self.sems.allocated().items():
            nm = getattr(h, "name", None) or str(k)
            name2sem[nm] = h
        drain_inst.ins.sync_info = mybir.SyncInfo(
            on_wait=[waits[0]], on_update=[]
        )
        for w in waits[1:]:
            h = name2sem.get(w.ant_name)
            assert h is not None, (w.ant_name, list(name2sem))
            n = nc.sync.nop(nofuse=True)
            n.wait_op(h, w.wait_value, "sem-ge")
    nc.all_engine_barrier()
    popped = nc._tile_sem_poison_stack.pop()
    assert popped is self._sem_poison
    nc.clear_and_free_semaphores(list(self.sems.allocated().values()))
    nc.all_engine_barrier()


tile.TileContext._drain_and_barrier = _patched_drain_and_barrier
bass_utils.upload_artifacts = lambda tmpdir: tmpdir

_legalize_counter = [0]


def legalize_waits(nc):
    n_split = 0
    for f in nc.m.functions:
        for bb in f.blocks:
            insts = bb.instructions
            if not any(
                i.sync_info is not None and len(i.sync_info.on_wait) > 1
                for i in insts
            ):
                continue
            new_list = []
            for ins in insts:
                si = ins.sync_info
                if si is not None and len(si.on_wait) > 1:
                    waits = list(si.on_wait)
                    for w in waits[:-1]:
                        _legalize_counter[0] += 1
                        nop = mybir.InstNoOp(
                            name=f"lw_nop_{_legalize_counter[0]}",
                            ins=[], outs=[],
                        )
                        nop.engine = ins.engine
                        nop.sync_info = mybir.SyncInfo(
                            on_wait=[w], on_update=[]
                        )
                        new_list.append(nop)
                        n_split += 1
                    ins.sync_info = mybir.SyncInfo(
                        on_wait=[waits[-1]], on_update=list(si.on_update)
                    )
                new_list.append(ins)
            bb.instructions = new_list
    return n_split


# ─────────────────────────────────────────────────────────────────────
# Kernel build
# ─────────────────────────────────────────────────────────────────────

F32 = mybir.dt.float32
F16 = mybir.dt.float16
F32R = mybir.dt.float32r
BF16 = mybir.dt.bfloat16

DIM = 1024
HD = 64
B = 2
N = 2048
BN = B * N
SCALE = HD ** -0.5
N_CORES = 8
KT = DIM // 128
MC_B = N // 512  # 4
NT_B = N // 128  # 16


def _build_nc():
    mm_dt = BF16
    p_dt = BF16
    nc = bass.Bass("TRN2", target_bir_lowering=False, debug=False,
                   num_devices=N_CORES)
    xT = nc.dram_tensor("xT", [DIM, BN], mm_dt, kind="ExternalInput")
    # host pre-arranges qkv weights to [p, kt*m] so the DMA moves
    # contiguous 2KB partition lines instead of scattered 256B ones
    wq = nc.dram_tensor("wq", [128, DIM], mm_dt, kind="ExternalInput")
    wk = nc.dram_tensor("wk", [128, DIM], mm_dt, kind="ExternalInput")
    wv = nc.dram_tensor("wv", [128, DIM], mm_dt, kind="ExternalInput")
    wp = nc.dram_tensor("wp", [128, DIM], mm_dt, kind="ExternalInput")
    out = nc.dram_tensor("out", [BN, DIM], BF16, kind="ExternalOutput")
    out_t = out.ap().rearrange("(c p) d -> p c d", p=128)

    with tile.TileContext(nc) as tc:
        with (
            tc.tile_pool(name="xp", bufs=1) as xp,
            tc.tile_pool(name="wpool", bufs=1) as wpool,
            tc.tile_pool(name="qk", bufs=1) as qk,
            tc.tile_pool(name="pt", bufs=8) as ptp,
            tc.tile_pool(name="small", bufs=2) as sp,
            tc.tile_pool(name="osb", bufs=2) as osbp,
            tc.tile_pool(name="ostage", bufs=3) as osp,
            tc.tile_pool(name="ps_s", bufs=2, space="PSUM") as ps_s,
            tc.tile_pool(name="ps_big", bufs=2, space="PSUM") as ps_big,
            tc.tile_pool(name="ps_o", bufs=2, space="PSUM") as ps_o,
        ):
            # engine warmups (run during the DMA fill): a tiny exp
            # preloads the ACT spline table; a zero-matmul burst opens
            # the HAM clock gate (1.2 -> 2.4 GHz) before real work
            we_in = sp.tile([128, 64], F32, tag="we_i", bufs=1)
            we_out = sp.tile([128, 64], BF16, tag="we_o", bufs=1)
            nc.vector.memset(we_in[:], 0.0)
            nc.scalar.activation(we_out[:], we_in[:],
                                 mybir.ActivationFunctionType.Exp,
                                 scale=SCALE)
            wdum = sp.tile([128, 128], mm_dt, tag="wdum", bufs=1)
            nc.vector.memset(wdum[:], 0.0)
            warm_ps = ps_big.tile([128, 512], F32, tag="big",
                                  name="warm_ps")
            for _ in range(24):
                nc.tensor.matmul(warm_ps[:, 0:128], wdum[:], wdum[:],
                                 start=True, stop=True)

            # weights first (gpsimd queues) so QKV isn't stuck behind
            # the 8MB xT transfer; xT split per (k-tile, 512-token chunk)
            # ordered chunk-major so the first QKV matmul can start after
            # ~1MB instead of the full 8MB
            wq_s = wpool.tile([128, KT, 128], mm_dt, tag="wq")
            wk_s = wpool.tile([128, KT, 128], mm_dt, tag="wk")
            wv_s = wpool.tile([128, KT, 128], p_dt, tag="wv")
            for w_d, w_s in ((wq, wq_s), (wk, wk_s), (wv, wv_s)):
                nc.gpsimd.dma_start(
                    out=w_s[:],
                    in_=w_d.ap().rearrange("p (kt m) -> p kt m", kt=KT),
                )
            wp_s = wpool.tile([128, DIM], mm_dt, tag="wp")
            nc.gpsimd.dma_start(out=wp_s[:], in_=wp.ap())
            xT_s = xp.tile([128, KT, BN], mm_dt, tag="xT")
            xT_t = xT.ap().rearrange("(kt p) m -> p kt m", p=128)
            nc.sync.dma_start(out=xT_s[:, 0, 0:512],
                              in_=xT_t[:, 0, 0:512])
            nc.sync.dma_start(out=xT_s[:, 1, 0:512],
                              in_=xT_t[:, 1, 0:512])
            nc.sync.dma_start(out=xT_s[:, 2:4, 0:512],
                              in_=xT_t[:, 2:4, 0:512])
            nc.sync.dma_start(out=xT_s[:, 4:6, 0:512],
                              in_=xT_t[:, 4:6, 0:512])
            nc.sync.dma_start(out=xT_s[:, 6:KT, 0:512],
                              in_=xT_t[:, 6:KT, 0:512])
            for mc in range(1, 4):
                nc.sync.dma_start(
                    out=xT_s[:, :, ts(mc, 512)],
                    in_=xT_t[:, :, ts(mc, 512)],
                )
            nc.sync.dma_start(
                out=xT_s[:, :, N:BN],
                in_=xT_t[:, :, N:BN],
            )

            # ---- QKV projection work items ----
            QT_s = qk.tile([128, BN], mm_dt, tag="QT")
            KT_s = qk.tile([128, BN], mm_dt, tag="KT")
            V_s = qk.tile([128, 32, 2, 72], p_dt, tag="V")
            nc.vector.memset(V_s[:, :, :, 64], 1.0)

            def qkv_qt_kt(w_s, dst, mc, act_copy=False):
                ps = ps_big.tile([128, 512], F32, tag="big", name="qk_ps")
                for kt in range(KT):
                    nc.tensor.matmul(
                        ps[:], w_s[:, kt, :], xT_s[:, kt, ts(mc, 512)],
                        start=(kt == 0), stop=(kt == KT - 1),
                    )
                # groups dripped into b=0 units copy via ACT (exp has
                # slack there), unclogging the DVE queue that gates
                # ps_big rotation for the V groups
                if act_copy:
                    nc.scalar.copy(dst[:, ts(mc, 512)], ps[:])
                else:
                    nc.vector.tensor_copy(dst[:, ts(mc, 512)], ps[:])

            def qkv_v(mt):
                ps = ps_big.tile([128, 512], F32, tag="big", name="v_ps")
                for kt in range(KT):
                    nc.tensor.matmul(
                        ps[:, 0:128], xT_s[:, kt, ts(mt, 128)],
                        wv_s[:, kt, :],
                        start=(kt == 0), stop=(kt == KT - 1),
                    )
                nc.vector.tensor_copy(V_s[:, mt, 0, 0:64], ps[:, 0:64])
                nc.vector.tensor_copy(V_s[:, mt, 1, 0:64], ps[:, 64:128])

            def item_kt(mc):
                return lambda: qkv_qt_kt(wk_s, KT_s, mc, act_copy=True)

            def item_qt(mc):
                return lambda: qkv_qt_kt(wq_s, QT_s, mc,
                                         act_copy=(mc <= 4))

            def item_v(mt):
                return lambda: qkv_v(mt)

            # drip-feed schedule: QKV groups interleave with attention
            # chunks so the PE stream stays dense without bursts
            pre = [[[] for _ in range(16)] for _ in range(16)]
            pre[0][0] = [item_kt(0), item_qt(0), item_v(0)]
            for nt in range(15):
                pre[0][nt].append(item_v(nt + 1))
            pre[0][2].append(item_kt(1))
            pre[0][6].append(item_kt(2))
            pre[0][10].append(item_kt(3))
            for u in (1, 2, 3, 4, 5, 6, 7):
                pre[u - 1][8].append(item_qt(u))
            pre[1][0].append(item_kt(4))
            pre[1][2].append(item_kt(5))
            pre[2][0].append(item_kt(6))
            pre[2][2].append(item_kt(7))
            for i in range(8):
                pre[3][(i // 2) * 2 + 4].append(item_v(16 + i))
            for i in range(8):
                pre[4][(i // 2) * 2].append(item_v(24 + i))

            # ---- attention ----
            # ones64: K=64 stationary for the per-head normalization
            # broadcast (partition bases must be 32-aligned, so the den
            # rows live at {0, 32}); row 0 -> out cols 0-63 (head 0),
            # row 32 -> out cols 64-127 (head 1), rest zero
            ones64_b = sp.tile([64, 128], mm_dt, tag="ones64", bufs=1)
            nc.vector.memset(ones64_b[:], 0.0)
            nc.vector.memset(ones64_b[0:1, 0:64], 1.0)
            nc.vector.memset(ones64_b[32:33, 64:128], 1.0)

            def norm_and_proj_stages(q_off, w, den_s, o_list,
                                     tail=False, act_copies=False):
                """9 closures emitted at nt slots of the NEXT unit so
                each small PE/DVE block finds its inputs ready."""
                state = {}

                def st_copies():
                    # pack both heads' O^T into one [128, w] SBUF
                    # tile (rows 0-63 head0, 64-127 head1) + den rows
                    o_sb = osbp.tile([128, 512], F32, tag="osb",
                                     name="o_sb")
                    for h in range(2):
                        if tail or act_copies:
                            nc.scalar.copy(o_sb[ts(h, 64), 0:w],
                                           o_list[h][0:64, 0:w])
                        else:
                            nc.vector.tensor_copy(o_sb[ts(h, 64), 0:w],
                                                  o_list[h][0:64, 0:w])
                        nc.vector.tensor_copy(
                            den_s[32 * h:32 * h + 1, 0:w],
                            o_list[h][64:65, 0:w])
                    state["o_sb"] = o_sb

                def mk_recip(half):
                    # w=512: two halves so other DVE work interleaves;
                    # w=256: one call (DVE recip has a ~1.2us floor, so
                    # halving the width doesn't halve the time)
                    def st_recip():
                        if "r" not in state:
                            state["r"] = sp.tile([64, 512], F32,
                                                 tag="rall",
                                                 name="r_all")
                            nc.vector.reciprocal(
                                state["r"][:, 0:min(w, 256)],
                                den_s[:, 0:min(w, 256)])
                        elif w == 512:
                            nc.vector.reciprocal(state["r"][:, 256:512],
                                                 den_s[:, 256:512])
                    return st_recip

                def st_rmat():
                    rb = sp.tile([64, 512], mm_dt, tag="rb", name="r_bf")
                    nc.vector.tensor_copy(rb[:, 0:w], state["r"][:, 0:w])
                    R_ps = ps_big.tile([128, 512], F32, tag="big",
                                       name="R_ps")
                    nc.tensor.matmul(
                        R_ps[:, 0:w], ones64_b[:], rb[:, 0:w],
                        start=True, stop=True,
                    )
                    state["R"] = R_ps

                def st_mul():
                    AT = sp.tile([128, 512], mm_dt, tag="AT", name="AT_s")
                    nc.vector.tensor_mul(AT[:, 0:w], state["o_sb"][:, 0:w],
                                         state["R"][:, 0:w])
                    state["AT"] = AT

                def mk_proj(mt):
                    def st_proj():
                        if "out" not in state:
                            state["out"] = osp.tile(
                                [128, 4, DIM], BF16, tag="out",
                                name="out_stage",
                            )
                        for cc in range(2):
                            p_ps = ps_big.tile([128, 512], F32, tag="big")
                            nc.tensor.matmul(
                                p_ps[:], state["AT"][:, ts(mt, 128)],
                                wp_s[:, ts(cc, 512)],
                                start=True, stop=True,
                            )
                            if tail:
                                nc.scalar.copy(
                                    state["out"][:, mt, ts(cc, 512)],
                                    p_ps[:])
                            else:
                                nc.vector.tensor_copy(
                                    state["out"][:, mt, ts(cc, 512)],
                                    p_ps[:])
                            if tail:
                                for q4 in range(2 * cc, 2 * cc + 2):
                                    eng = (nc.sync if q4 % 2 == 0
                                           else nc.gpsimd)
                                    eng.dma_start(
                                        out=out_t[:, q_off // 128 + mt,
                                                  ts(q4, 256)],
                                        in_=state["out"][:, mt,
                                                         ts(q4, 256)],
                                    )
                        if not tail:
                            eng = nc.sync if mt % 2 == 0 else nc.gpsimd
                            eng.dma_start(
                                out=out_t[:, q_off // 128 + mt, :],
                                in_=state["out"][:, mt, :],
                            )
                    return st_proj

                return [st_copies, mk_recip(0), mk_recip(1), st_rmat,
                        st_mul] + [mk_proj(mt) for mt in range(w // 128)]

            pending = None
            av_q = []  # AV emission runs 2 nt behind scores/exp so the
            # in-order PE queue never stalls waiting on the ACT exp

            # last 512-query unit split into two 256-query halves so its
            # norm+proj chain overlaps the second half instead of
            # serializing into the kernel tail
            units = []
            for b in range(B):
                for mc in range(MC_B):
                    u = b * MC_B + mc
                    if u < 7:
                        units.append((b, u, u * 512, 512))
                    else:
                        units.append((b, u, u * 512, 256))
                        units.append((b, u, u * 512 + 256, 256))

            for b, unit, q_off, w in units:
                # den rows {0, 32}; memset (on the idle gpsimd
                # engine) so the unused rows can't feed inf/nan into
                # the bf16 broadcast matmul
                den_s = sp.tile([64, 512], F32, tag="den")
                nc.gpsimd.memset(den_s[:], 1.0)
                o_list = [ps_o.tile([128, 512], F32, tag="o",
                                    name=f"o_ps_{h}")
                          for h in range(2)]
                for nt in range(NT_B):
                    for fn in pre[unit][nt] if q_off % 512 == 0 else []:
                        fn()
                    if nt in (2, 3, 4, 9, 10, 11, 12, 13, 14) \
                            and pending:
                        pending.pop(0)()
                    # both heads' score matmuls adjacent: disjoint PE
                    # row groups run concurrently
                    s_ps = ps_s.tile([128, 2, 512], F32, tag="S")
                    for h in range(2):
                        h_sl = ts(h, 64)
                        nc.tensor.matmul(
                            s_ps[:, h, 0:w],
                            KT_s[h_sl, b * N + nt * 128:
                                 b * N + (nt + 1) * 128],
                            QT_s[h_sl, q_off:q_off + w],
                            start=True, stop=True,
                        )
                    PT_s = ptp.tile([128, 2, 512], p_dt, tag="PT")
                    nc.scalar.activation(
                        PT_s[:, :, 0:w], s_ps[:, :, 0:w],
                        mybir.ActivationFunctionType.Exp,
                        scale=SCALE,
                    )

                    def mk_av(o_l, bb, ntt, pt, ww):
                        def av():
                            for h in range(2):
                                nc.tensor.matmul(
                                    o_l[h][0:65, 0:ww],
                                    V_s[:, bb * NT_B + ntt, h, 0:65],
                                    pt[:, h, 0:ww],
                                    start=(ntt == 0),
                                    stop=(ntt == NT_B - 1),
                                )
                        return av

                    av_q.append(mk_av(o_list, b, nt, PT_s, w))
                    if len(av_q) > 2:
                        av_q.pop(0)()
                # the unit's last two AVs drain at the next unit's
                # nt=0/1; the o_sb/den copies are the first pending
                # stage (popped at nt=2, after both AVs)
                pending = norm_and_proj_stages(
                    q_off, w, den_s, o_list,
                    tail=(q_off + w == BN),
                    act_copies=(q_off < 3 * 512))
            while av_q:
                av_q.pop(0)()
            # dummy matmuls into the now-idle score pool keep the HAM
            # clock gate open across the tail's DVE-only recip window
            wt_ps = ps_s.tile([128, 2, 512], F32, tag="S",
                              name="tail_warm")
            for _ in range(40):
                nc.tensor.matmul(wt_ps[:, 0, 0:128], wdum[:], wdum[:],
                                 start=True, stop=True)
            for fn in pending:
                fn()
    legalize_waits(nc)
    return nc


_CACHE = {}


def _get_nc():
    if "nc" not in _CACHE:
        _CACHE["nc"] = _build_nc()
    return _CACHE["nc"]


def wpack_test(w):
    # [DIM, 128] -> [128p, KT*128] so each SBUF partition line is one
    # contiguous 2KB DMA read
    return np.ascontiguousarray(
        np.asarray(w, dtype=np.float32)
        .reshape(KT, 128, 128).transpose(1, 0, 2).reshape(128, DIM)
    ).astype(ml_dtypes.bfloat16)


def kernel(x, w_qkv, w_proj, b_proj):
    x = np.asarray(x, dtype=np.float32)
    w_qkv = np.asarray(w_qkv, dtype=np.float32)
    w_proj = np.asarray(w_proj, dtype=np.float32)
    b_proj = np.asarray(b_proj, dtype=np.float32)

    nc = _get_nc()
    bf = ml_dtypes.bfloat16

    xT = np.ascontiguousarray(x.reshape(BN, DIM).T).astype(bf)
    in_maps = []
    for c in range(N_CORES):
        sl = slice(128 * c, 128 * (c + 1))
        in_maps.append({
            "xT": xT,
            "wq": wpack_test(w_qkv[:, sl]),
            "wk": wpack_test(w_qkv[:, DIM + 128 * c:DIM + 128 * (c + 1)]),
            "wv": wpack_test(
                w_qkv[:, 2 * DIM + 128 * c:2 * DIM + 128 * (c + 1)]),
            "wp": np.ascontiguousarray(w_proj[sl, :]).astype(bf),
        })
    res = run_bass_kernel_spmd(nc, in_maps, list(range(N_CORES)),
                               trace=False)
    acc = res.results[0]["out"].astype(np.float32).copy()
    for c in range(1, N_CORES):
        acc += res.results[c]["out"]
    acc += b_proj[None, :]
    return acc.reshape(B, N, DIM)

